# revision 1
# baseline (speedup 1.0000x reference)
"""MoE grouped-GEMM (8 experts) on 8 Trainium2 NeuronCores.

Problem: input [32768, 1024] routed contiguously to 8 experts (counts in
num_experts_per_token); expert i computes x_i @ W_i.T + b_i with
W [8, 4096, 1024], b [8, 4096]. Output [32768, 4096].

Sharding: expert-parallel, expert i <-> core i. Zero collectives: the host
slices each expert's token block, packs x and W into the exact SBUF tile
layout (contraction dim DIN on partitions, k-blocks as column groups) so
every input DMA is one fully-contiguous 2 MB linear read, each core runs a
4096x1024x4096 GEMM (+bias), and the host concatenates per-core outputs.

Device kernel (per core), all matmuls float32r (1 cycle/row at N=512 vs 4
for plain fp32; rel err ~1.5e-4 with fp32 PSUM accumulation):
  - wP [8n, 128, 8k*512] resident weights, streamed n-block-wise just in
    time (n=0 gates, n=1..3 land during chunk 0, n=4..7 mid-phase-0)
  - xP [8c, 128, 8k*512] activations streamed per 512-token chunk on the
    scalar HWDGE ring (parallel to weight loads on the sync ring)
  - two n-phases (n0-3, n4-7) re-stream x so no chunk window's HBM demand
    exceeds the ~358 GB/s per-core cap
  - PSUM k-accumulation over 8 banks; DVE fuses bias-add with PSUM drain
"""

import sys

if "/opt/trn_rl_repo" not in sys.path:
    sys.path.insert(0, "/opt/trn_rl_repo")

import numpy as np

E, T, DIN, DOUT = 8, 32768, 1024, 4096
NCORES = 8
TOKC = T // NCORES  # tokens per core (capacity)

KT = 128   # contraction tile (SBUF partitions)
MT = 128   # token tile (PSUM partitions)
NT = 512   # dout tile (one fp32 PSUM bank)
KTILES = DIN // KT    # 8
MTILES = TOKC // MT   # 32
NTILES = DOUT // NT   # 8

CT = 512                 # token chunk (streamed x granularity)
CHUNKS = TOKC // CT      # 8
CMT = CT // MT           # 4 token tiles per chunk

_CACHE = {}


def _build_nc():
    import concourse.bacc as bacc
    import concourse.tile as tile
    import concourse.mybir as mybir

    nc = bacc.Bacc("TRN2", target_bir_lowering=False, debug=False,
                   num_devices=NCORES)

    xP = nc.dram_tensor("xP", [CHUNKS, KT, KTILES * CT], mybir.dt.float32r,
                        kind="ExternalInput")
    wP = nc.dram_tensor("wP", [NTILES, KT, KTILES * NT], mybir.dt.float32r,
                        kind="ExternalInput")
    bias_b = nc.dram_tensor("bias_b", [MT, DOUT], mybir.dt.float32,
                            kind="ExternalInput")
    y = nc.dram_tensor("y", [TOKC, DOUT], mybir.dt.float32,
                       kind="ExternalOutput")

    with tile.TileContext(nc) as tc:
        with (
            tc.tile_pool(name="xpool", bufs=2) as xpool,
            tc.tile_pool(name="wpool", bufs=1) as wpool,
            tc.tile_pool(name="bpool", bufs=1) as bpool,
            tc.tile_pool(name="opool", bufs=9) as opool,
            tc.tile_pool(name="psum", bufs=8, space="PSUM") as psum_pool,
        ):
            # resident weights: one [128, 4096] tile per n-block, each a
            # single contiguous 2 MB DMA. n=0 first (compute gate).
            wt = [wpool.tile([KT, KTILES * NT], mybir.dt.float32r,
                             name=f"wt{n}", tag=f"wt{n}")
                  for n in range(NTILES)]
            # gate tiles load k-sliced so the first matmuls start on the
            # first 256 KB slice instead of the whole 2 MB tile
            for k in range(KTILES):
                nc.sync.dma_start(wt[0][:, k * NT:(k + 1) * NT],
                                  wP[0][:, k * NT:(k + 1) * NT])

            def load_chunk(c, sliced=False):
                # scalar (ACT) HWDGE ring: parallel to weight loads on sync
                t = xpool.tile([KT, KTILES * CT], mybir.dt.float32r,
                               name="xtc", tag="xtc")
                if sliced:
                    for k in range(KTILES):
                        nc.scalar.dma_start(t[:, k * CT:(k + 1) * CT],
                                            xP[c][:, k * CT:(k + 1) * CT])
                else:
                    nc.scalar.dma_start(t[:], xP[c])
                return t

            xtc_cur = load_chunk(0, sliced=True)

            # wT n=1..3 just in time for chunk-0 compute; bias (needed
            # ~20us in) rides after n=1. n=4..7 load mid-phase-0.
            NPH = NTILES // 2  # n-blocks per phase
            bias_t = bpool.tile([MT, DOUT], mybir.dt.float32,
                                name="bias_t", tag="bias_t")
            for n in range(1, NPH):
                nc.sync.dma_start(wt[n][:], wP[n])
                if n == 1:
                    nc.sync.dma_start(bias_t[:], bias_b[:])

            for phase in range(2):
                for c in range(CHUNKS):
                    if phase == 0 and c == 3:
                        for n in range(NPH, NTILES):
                            nc.sync.dma_start(wt[n][:], wP[n])
                    last = phase == 1 and c == CHUNKS - 1
                    xtc_next = None if last else load_chunk((c + 1) % CHUNKS)
                    for n in range(phase * NPH, (phase + 1) * NPH):
                        for m in range(CMT):
                            acc = psum_pool.tile([MT, NT], mybir.dt.float32,
                                                 name="acc", tag="acc")
                            for k in range(KTILES):
                                nc.tensor.matmul(
                                    acc[:],
                                    xtc_cur[:, k * CT + m * MT:
                                            k * CT + (m + 1) * MT],
                                    wt[n][:, k * NT:(k + 1) * NT],
                                    start=(k == 0), stop=(k == KTILES - 1))
                            ot = opool.tile([MT, NT], mybir.dt.float32,
                                            name="ot", tag="ot")
                            nc.vector.tensor_add(
                                ot[:], acc[:], bias_t[:, n * NT:(n + 1) * NT])
                            row0 = c * CT + m * MT
                            nc.scalar.dma_start(
                                y[row0:row0 + MT, n * NT:(n + 1) * NT], ot[:])
                    xtc_cur = xtc_next

    nc.compile()
    return nc


def _install_neff_cache():
    """Disk-cache walrus NEFF compiles keyed on the BIR bytes."""
    if _CACHE.get("neff_cache_installed"):
        return
    _CACHE["neff_cache_installed"] = True
    import hashlib
    import os
    import shutil

    import concourse.bass2jax as bass2jax

    cache_dir = "/root/.neff_bir_cache"
    os.makedirs(cache_dir, exist_ok=True)
    orig = bass2jax.compile_bir_kernel

    def cached_compile(ant_bir_str, tmpdir, neff_name="file.neff", **kw):
        key = hashlib.sha256(
            ant_bir_str if isinstance(ant_bir_str, bytes)
            else ant_bir_str.encode()).hexdigest()
        hit = os.path.join(cache_dir, key + ".neff")
        dst = os.path.join(tmpdir, neff_name)
        if os.path.exists(hit):
            shutil.copyfile(hit, dst)
            return dst
        out = orig(ant_bir_str, tmpdir, neff_name=neff_name, **kw)
        try:
            shutil.copyfile(out, hit)
        except OSError:
            pass
        return out

    bass2jax.compile_bir_kernel = cached_compile


def _get_nc():
    if "nc" not in _CACHE:
        _install_neff_cache()
        _CACHE["nc"] = _build_nc()
    return _CACHE["nc"]


def _pack_blocks(a2d, blocks, inner):
    """[R*inner? no: (blocks*inner, K*128)] -> [blocks, 128, K*inner]:
    out[b, p, k*inner + j] = a2d[b*inner + j, k*128 + p]."""
    rows, cols = a2d.shape
    kb = cols // KT
    return np.ascontiguousarray(
        a2d.reshape(blocks, inner, kb, KT).transpose(0, 3, 2, 1)
        .reshape(blocks, KT, kb * inner))


def kernel(input, weight, bias, num_experts_per_token):
    from concourse.bass_utils import run_bass_kernel_spmd

    input = np.ascontiguousarray(np.asarray(input, dtype=np.float32))
    weight = np.ascontiguousarray(np.asarray(weight, dtype=np.float32))
    bias = np.ascontiguousarray(np.asarray(bias, dtype=np.float32))
    counts = np.asarray(num_experts_per_token).astype(np.int64)
    offsets = np.concatenate([[0], np.cumsum(counts)]).astype(np.int64)

    if counts.max() > TOKC:
        # capacity overflow (never hit with balanced routing): numpy fallback
        outs = []
        for i in range(E):
            xi = input[offsets[i]:offsets[i + 1]]
            outs.append(xi @ weight[i].T + bias[i])
        return np.concatenate(outs, axis=0)

    in_maps = []
    for i in range(E):
        xi = input[offsets[i]:offsets[i + 1]]  # [n_i, DIN]
        if xi.shape[0] < TOKC:
            xi = np.concatenate(
                [xi, np.zeros((TOKC - xi.shape[0], DIN), np.float32)], axis=0)
        xp = _pack_blocks(xi, CHUNKS, CT)          # [8, 128, 4096]
        wp = _pack_blocks(weight[i], NTILES, NT)   # [8, 128, 4096]
        bb = np.ascontiguousarray(
            np.broadcast_to(bias[i][None, :], (MT, DOUT)))
        in_maps.append({"xP": xp, "wP": wp, "bias_b": bb})

    nc = _get_nc()
    import os
    trace = bool(int(os.environ.get("KERNEL_TRACE", "0")))
    res = run_bass_kernel_spmd(nc, in_maps, core_ids=list(range(NCORES)),
                               trace=trace)
    _CACHE["last_result"] = res

    out = np.empty((T, DOUT), dtype=np.float32)
    pos = 0
    for i in range(E):
        n_i = int(counts[i])
        out[pos:pos + n_i] = res.results[i]["y"][:n_i]
        pos += n_i
    return out



# revision 4
# speedup vs baseline: 1.0310x; 1.0310x over previous
"""MoE grouped-GEMM (8 experts) on 8 Trainium2 NeuronCores.

Problem: input [32768, 1024] routed contiguously to 8 experts (counts in
num_experts_per_token); expert i computes x_i @ W_i.T + b_i with
W [8, 4096, 1024], b [8, 4096]. Output [32768, 4096].

Sharding: expert-parallel, expert i <-> core i. Zero collectives; the host
slices per-expert token blocks, packs x and W into SBUF-tile layout, each
core runs a 4096x1024x4096 GEMM (+bias), host concatenates outputs.

Per-core kernel, bf16 operands (rel err ~3e-4 with fp32 PSUM accumulation;
same 1 cycle/row PE throughput as fp32r but half the HBM/SBUF footprint and
cheap pipelineable weight loads):
  - x [128, 32768] and W [128, 32768] bf16 fully SBUF-resident: each is
    read from HBM exactly once (the fp32r version re-streamed x twice)
  - two n-super-phases (n0-3, n4-7) so only 4.2 MB of W gates startup;
    phase-0 W is split across the sync+vector queues, x streams m-major on
    the gpsimd queue, so the first m-tiles start within ~2 us
  - loop order m -> k -> n(4): 4 consecutive matmuls share one stationary
    x tile (LDWEIGHTS amortized), 4 PSUM banks per m-tile round-robin of 8,
    bank drains (DVE bias-add fused) stagger without bursting
  - outputs stream per [128,512] tile on the scalar queue (67 MB fp32)
"""

import sys

if "/opt/trn_rl_repo" not in sys.path:
    sys.path.insert(0, "/opt/trn_rl_repo")

import numpy as np

E, T, DIN, DOUT = 8, 32768, 1024, 4096
NCORES = 8
TOKC = T // NCORES  # tokens per core (capacity)

KT = 128   # contraction tile (SBUF partitions)
MT = 128   # token tile (PSUM partitions)
NT = 512   # dout tile (one fp32 PSUM bank)
KTILES = DIN // KT    # 8
MTILES = TOKC // MT   # 32
NTILES = DOUT // NT   # 8
NPH = 4               # n-tiles per super-phase
PHASES = NTILES // NPH  # 2
PHC = NPH * NT        # columns per (k, phase) W block: 2048
WPH = KTILES * PHC    # W columns per phase: 16384

_CACHE = {}


def _build_nc():
    import concourse.bacc as bacc
    import concourse.tile as tile
    import concourse.mybir as mybir

    nc = bacc.Bacc("TRN2", target_bir_lowering=False, debug=False,
                   num_devices=NCORES)

    # xD[p, m*1024 + k*128 + j] = x[m*128 + j, k*128 + p]   (bf16)
    xD = nc.dram_tensor("xD", [KT, MTILES * DIN], mybir.dt.bfloat16,
                        kind="ExternalInput")
    # wD[p, q*16384 + k*2048 + t*512 + j] = W[(q*4+t)*512 + j, k*128 + p]
    wD = nc.dram_tensor("wD", [KT, PHASES * WPH], mybir.dt.bfloat16,
                        kind="ExternalInput")
    bD = nc.dram_tensor("bD", [MT, DOUT], mybir.dt.float32,
                        kind="ExternalInput")
    y = nc.dram_tensor("y", [TOKC, DOUT], mybir.dt.float32,
                       kind="ExternalOutput")

    with tile.TileContext(nc) as tc:
        with (
            tc.tile_pool(name="xpool", bufs=1) as xpool,
            tc.tile_pool(name="wpool", bufs=1) as wpool,
            tc.tile_pool(name="bpool", bufs=1) as bpool,
            tc.tile_pool(name="opool", bufs=8) as opool,
            tc.tile_pool(name="psum", bufs=8, space="PSUM") as psum_pool,
        ):
            xS = xpool.tile([KT, MTILES * DIN], mybir.dt.bfloat16,
                            name="xS", tag="xS")
            wS = wpool.tile([KT, PHASES * WPH], mybir.dt.bfloat16,
                            name="wS", tag="wS")
            bS = bpool.tile([MT, DOUT], mybir.dt.float32,
                            name="bS", tag="bS")

            # --- streamed loads, ordered so compute gates release early ---
            # Only 3 DMA queues exist: sync (SP), scalar (Activation),
            # gpsimd (SWDGE). W phase 0 (4.2 MB) is split across sync
            # (k-even, k0 in 4 n-slices so the first matmul gates on
            # 128 KB) and scalar (k-odd, behind the phase-0 bias slice).
            for t in range(NPH):
                nc.sync.dma_start(wS[:, t * NT:(t + 1) * NT],
                                  wD[:, t * NT:(t + 1) * NT])
            nc.scalar.dma_start(bS[:, 0:NPH * NT], bD[:, 0:NPH * NT])
            for k in range(1, KTILES):
                eng = nc.sync if k % 2 == 0 else nc.scalar
                eng.dma_start(wS[:, k * PHC:(k + 1) * PHC],
                              wD[:, k * PHC:(k + 1) * PHC])
            # x (8.4 MB) m-major on gpsimd: first 4 m-tiles sliced, rest in
            # 4-m-tile chunks.
            for m in range(4):
                nc.gpsimd.dma_start(xS[:, m * DIN:(m + 1) * DIN],
                                    xD[:, m * DIN:(m + 1) * DIN])
            for c in range(1, 8):
                nc.gpsimd.dma_start(xS[:, c * 4 * DIN:(c + 1) * 4 * DIN],
                                    xD[:, c * 4 * DIN:(c + 1) * 4 * DIN])
            # phase-1 bias on scalar (needed only ~220 us in).
            nc.scalar.dma_start(bS[:, NPH * NT:DOUT], bD[:, NPH * NT:DOUT])
            # W phase 1 (4.2 MB) on sync, behind the phase-0 slices.
            nc.sync.dma_start(wS[:, WPH:WPH + WPH // 2],
                              wD[:, WPH:WPH + WPH // 2])
            nc.sync.dma_start(wS[:, WPH + WPH // 2:2 * WPH],
                              wD[:, WPH + WPH // 2:2 * WPH])

            for q in range(PHASES):
                for m in range(MTILES):
                    last = (q == PHASES - 1 and m == MTILES - 1)
                    accs = [psum_pool.tile([MT, NT], mybir.dt.float32,
                                           name="acc", tag="acc")
                            for _ in range(NPH)]

                    def mm(k, t):
                        nc.tensor.matmul(
                            accs[t][:],
                            xS[:, m * DIN + k * KT:m * DIN + (k + 1) * KT],
                            wS[:, q * WPH + k * PHC + t * NT:
                               q * WPH + k * PHC + (t + 1) * NT],
                            start=(k == 0), stop=(k == KTILES - 1))

                    if not last:
                        # k-outer: 4 matmuls share each stationary x tile
                        for k in range(KTILES):
                            for t in range(NPH):
                                mm(k, t)
                    else:
                        # n-outer on the final m-tile: banks stop early and
                        # drain staggered, shrinking the kernel tail
                        for t in range(NPH):
                            for k in range(KTILES):
                                mm(k, t)
                    for t in range(NPH):
                        n = q * NPH + t
                        ot = opool.tile([MT, NT], mybir.dt.float32,
                                        name="ot", tag="ot")
                        nc.vector.tensor_add(
                            ot[:], accs[t][:], bS[:, n * NT:(n + 1) * NT])
                        row0 = m * MT
                        nc.scalar.dma_start(
                            y[row0:row0 + MT, n * NT:(n + 1) * NT], ot[:])

    nc.compile()
    return nc


def _install_neff_cache():
    """Disk-cache walrus NEFF compiles keyed on the BIR bytes."""
    if _CACHE.get("neff_cache_installed"):
        return
    _CACHE["neff_cache_installed"] = True
    import hashlib
    import os
    import shutil

    import concourse.bass2jax as bass2jax

    cache_dir = "/root/.neff_bir_cache"
    os.makedirs(cache_dir, exist_ok=True)
    orig = bass2jax.compile_bir_kernel

    def cached_compile(ant_bir_str, tmpdir, neff_name="file.neff", **kw):
        key = hashlib.sha256(
            ant_bir_str if isinstance(ant_bir_str, bytes)
            else ant_bir_str.encode()).hexdigest()
        hit = os.path.join(cache_dir, key + ".neff")
        dst = os.path.join(tmpdir, neff_name)
        if os.path.exists(hit):
            shutil.copyfile(hit, dst)
            return dst
        out = orig(ant_bir_str, tmpdir, neff_name=neff_name, **kw)
        try:
            shutil.copyfile(out, hit)
        except OSError:
            pass
        return out

    bass2jax.compile_bir_kernel = cached_compile


def _get_nc():
    if "nc" not in _CACHE:
        _install_neff_cache()
        _CACHE["nc"] = _build_nc()
    return _CACHE["nc"]


def kernel(input, weight, bias, num_experts_per_token):
    import ml_dtypes
    from concourse.bass_utils import run_bass_kernel_spmd

    bf16 = ml_dtypes.bfloat16
    input = np.ascontiguousarray(np.asarray(input, dtype=np.float32))
    weight = np.ascontiguousarray(np.asarray(weight, dtype=np.float32))
    bias = np.ascontiguousarray(np.asarray(bias, dtype=np.float32))
    counts = np.asarray(num_experts_per_token).astype(np.int64)
    offsets = np.concatenate([[0], np.cumsum(counts)]).astype(np.int64)

    if counts.max() > TOKC:
        # capacity overflow (never hit with balanced routing): numpy fallback
        outs = []
        for i in range(E):
            xi = input[offsets[i]:offsets[i + 1]]
            outs.append(xi @ weight[i].T + bias[i])
        return np.concatenate(outs, axis=0)

    in_maps = []
    for i in range(E):
        xi = input[offsets[i]:offsets[i + 1]]  # [n_i, DIN]
        if xi.shape[0] < TOKC:
            xi = np.concatenate(
                [xi, np.zeros((TOKC - xi.shape[0], DIN), np.float32)], axis=0)
        # [p, m*1024 + k*128 + j] = x[m*128+j, k*128+p]
        xq = np.ascontiguousarray(
            xi.astype(bf16).reshape(MTILES, MT, KTILES, KT)
            .transpose(3, 0, 2, 1).reshape(KT, MTILES * DIN))
        # [p, q*16384 + k*2048 + t*512 + j] = W[(q*4+t)*512+j, k*128+p]
        wq = np.ascontiguousarray(
            weight[i].astype(bf16).reshape(PHASES, NPH, NT, KTILES, KT)
            .transpose(4, 0, 3, 1, 2).reshape(KT, PHASES * WPH))
        bb = np.ascontiguousarray(
            np.broadcast_to(bias[i][None, :], (MT, DOUT)))
        in_maps.append({"xD": xq, "wD": wq, "bD": bb})

    nc = _get_nc()
    import os
    trace = bool(int(os.environ.get("KERNEL_TRACE", "0")))
    res = run_bass_kernel_spmd(nc, in_maps, core_ids=list(range(NCORES)),
                               trace=trace)
    _CACHE["last_result"] = res

    out = np.empty((T, DOUT), dtype=np.float32)
    pos = 0
    for i in range(E):
        n_i = int(counts[i])
        out[pos:pos + n_i] = res.results[i]["y"][:n_i]
        pos += n_i
    return out


# revision 6
# speedup vs baseline: 1.0425x; 1.0112x over previous
"""MoE grouped-GEMM (8 experts) on 8 Trainium2 NeuronCores.

Problem: input [32768, 1024] routed contiguously to 8 experts (counts in
num_experts_per_token); expert i computes x_i @ W_i.T + b_i with
W [8, 4096, 1024], b [8, 4096]. Output [32768, 4096].

Sharding: expert-parallel, expert i <-> core i. Zero collectives; the host
slices per-expert token blocks, packs x and W into SBUF-tile layout, each
core runs a 4096x1024x4096 GEMM (+bias), host concatenates outputs.

Per-core kernel, bf16 operands (rel err ~3e-4 with fp32 PSUM accumulation;
same 1 cycle/row PE throughput as fp32r but half the HBM/SBUF footprint and
cheap pipelineable weight loads):
  - x [128, 32768] and W [128, 32768] bf16 fully SBUF-resident: each is
    read from HBM exactly once (the fp32r version re-streamed x twice)
  - two n-super-phases (n0-3, n4-7) so only 4.2 MB of W gates startup;
    phase-0 W is split across the sync+vector queues, x streams m-major on
    the gpsimd queue, so the first m-tiles start within ~2 us
  - loop order m -> k -> n(4): 4 consecutive matmuls share one stationary
    x tile (LDWEIGHTS amortized), 4 PSUM banks per m-tile round-robin of 8,
    bank drains (DVE bias-add fused) stagger without bursting
  - outputs stream per [128,512] tile on the scalar queue (67 MB fp32)
"""

import sys

if "/opt/trn_rl_repo" not in sys.path:
    sys.path.insert(0, "/opt/trn_rl_repo")

import numpy as np

E, T, DIN, DOUT = 8, 32768, 1024, 4096
NCORES = 8
TOKC = T // NCORES  # tokens per core (capacity)

KT = 128   # contraction tile (SBUF partitions)
MT = 128   # token tile (PSUM partitions)
NT = 512   # dout tile (one fp32 PSUM bank)
KTILES = DIN // KT    # 8
MTILES = TOKC // MT   # 32
NTILES = DOUT // NT   # 8
NPH = 4               # n-tiles per super-phase
PHASES = NTILES // NPH  # 2
PHC = NPH * NT        # columns per (k, phase) W block: 2048
WPH = KTILES * PHC    # W columns per phase: 16384

_CACHE = {}


def _build_nc():
    import concourse.bacc as bacc
    import concourse.tile as tile
    import concourse.mybir as mybir

    nc = bacc.Bacc("TRN2", target_bir_lowering=False, debug=False,
                   num_devices=NCORES)

    # xD[p, m*1024 + k*128 + j] = x[m*128 + j, k*128 + p]   (bf16)
    xD = nc.dram_tensor("xD", [KT, MTILES * DIN], mybir.dt.bfloat16,
                        kind="ExternalInput")
    # wD[p, q*16384 + k*2048 + t*512 + j] = W[(q*4+t)*512 + j, k*128 + p]
    wD = nc.dram_tensor("wD", [KT, PHASES * WPH], mybir.dt.bfloat16,
                        kind="ExternalInput")
    bD = nc.dram_tensor("bD", [MT, DOUT], mybir.dt.float32,
                        kind="ExternalInput")
    y = nc.dram_tensor("y", [TOKC, DOUT], mybir.dt.float32,
                       kind="ExternalOutput")

    with tile.TileContext(nc) as tc:
        with (
            tc.tile_pool(name="xpool", bufs=1) as xpool,
            tc.tile_pool(name="wpool", bufs=1) as wpool,
            tc.tile_pool(name="bpool", bufs=1) as bpool,
            tc.tile_pool(name="opool", bufs=8) as opool,
            tc.tile_pool(name="psum", bufs=8, space="PSUM") as psum_pool,
        ):
            xS = xpool.tile([KT, MTILES * DIN], mybir.dt.bfloat16,
                            name="xS", tag="xS")
            wS = wpool.tile([KT, PHASES * WPH], mybir.dt.bfloat16,
                            name="wS", tag="wS")
            bS = bpool.tile([MT, DOUT], mybir.dt.float32,
                            name="bS", tag="bS")

            # --- streamed loads, ordered so compute gates release early ---
            # Only 3 DMA queues exist: sync (SP HWDGE), scalar (Activation
            # HWDGE), gpsimd (SWDGE, slow issue). Queues spin up ~9 us in;
            # everything the first ~30 us of compute needs rides the front
            # of the two HWDGE queues, deferred freight rides gpsimd.
            # sync: x m0 gate, W phase-0 k-even (k0 n-sliced), then the
            # rest of x in 4-m-tile chunks.
            nc.sync.dma_start(xS[:, 0:DIN], xD[:, 0:DIN])
            for t in range(NPH):
                nc.sync.dma_start(wS[:, t * NT:(t + 1) * NT],
                                  wD[:, t * NT:(t + 1) * NT])
            for k in range(2, KTILES, 2):
                nc.sync.dma_start(wS[:, k * PHC:(k + 1) * PHC],
                                  wD[:, k * PHC:(k + 1) * PHC])
            for c in range(1, 8):
                nc.sync.dma_start(xS[:, c * 4 * DIN:(c + 1) * 4 * DIN],
                                  xD[:, c * 4 * DIN:(c + 1) * 4 * DIN])
            # scalar: W phase-0 k-odd, phase-0 bias, then outputs (below).
            for k in range(1, KTILES, 2):
                nc.scalar.dma_start(wS[:, k * PHC:(k + 1) * PHC],
                                    wD[:, k * PHC:(k + 1) * PHC])
            nc.scalar.dma_start(bS[:, 0:NPH * NT], bD[:, 0:NPH * NT])
            # gpsimd: x m1-3 gates, then W phase 1 + phase-1 bias (all
            # needed only ~200+ us in).
            for m in range(1, 4):
                nc.gpsimd.dma_start(xS[:, m * DIN:(m + 1) * DIN],
                                    xD[:, m * DIN:(m + 1) * DIN])
            nc.gpsimd.dma_start(wS[:, WPH:WPH + WPH // 2],
                                wD[:, WPH:WPH + WPH // 2])
            nc.gpsimd.dma_start(wS[:, WPH + WPH // 2:2 * WPH],
                                wD[:, WPH + WPH // 2:2 * WPH])
            nc.gpsimd.dma_start(bS[:, NPH * NT:DOUT], bD[:, NPH * NT:DOUT])

            for q in range(PHASES):
                for m in range(MTILES):
                    last = (q == PHASES - 1 and m == MTILES - 1)
                    accs = [psum_pool.tile([MT, NT], mybir.dt.float32,
                                           name="acc", tag="acc")
                            for _ in range(NPH)]

                    def mm(k, t):
                        nc.tensor.matmul(
                            accs[t][:],
                            xS[:, m * DIN + k * KT:m * DIN + (k + 1) * KT],
                            wS[:, q * WPH + k * PHC + t * NT:
                               q * WPH + k * PHC + (t + 1) * NT],
                            start=(k == 0), stop=(k == KTILES - 1))

                    if not last:
                        # k-outer: 4 matmuls share each stationary x tile
                        for k in range(KTILES):
                            for t in range(NPH):
                                mm(k, t)
                    else:
                        # n-outer on the final m-tile: banks stop early and
                        # drain staggered, shrinking the kernel tail
                        for t in range(NPH):
                            for k in range(KTILES):
                                mm(k, t)
                    for t in range(NPH):
                        n = q * NPH + t
                        ot = opool.tile([MT, NT], mybir.dt.float32,
                                        name="ot", tag="ot")
                        nc.vector.tensor_add(
                            ot[:], accs[t][:], bS[:, n * NT:(n + 1) * NT])
                        row0 = m * MT
                        # last two m-tiles flush on the (idle) sync queue
                        # so the kernel tail isn't gated on the long
                        # scalar output backlog
                        oeng = nc.sync if (q == PHASES - 1
                                           and m >= MTILES - 2) else nc.scalar
                        oeng.dma_start(
                            y[row0:row0 + MT, n * NT:(n + 1) * NT], ot[:])

    nc.compile()
    return nc


def _install_neff_cache():
    """Disk-cache walrus NEFF compiles keyed on the BIR bytes."""
    if _CACHE.get("neff_cache_installed"):
        return
    _CACHE["neff_cache_installed"] = True
    import hashlib
    import os
    import shutil

    import concourse.bass2jax as bass2jax

    cache_dir = "/root/.neff_bir_cache"
    os.makedirs(cache_dir, exist_ok=True)
    orig = bass2jax.compile_bir_kernel

    def cached_compile(ant_bir_str, tmpdir, neff_name="file.neff", **kw):
        key = hashlib.sha256(
            ant_bir_str if isinstance(ant_bir_str, bytes)
            else ant_bir_str.encode()).hexdigest()
        hit = os.path.join(cache_dir, key + ".neff")
        dst = os.path.join(tmpdir, neff_name)
        if os.path.exists(hit):
            shutil.copyfile(hit, dst)
            return dst
        out = orig(ant_bir_str, tmpdir, neff_name=neff_name, **kw)
        try:
            shutil.copyfile(out, hit)
        except OSError:
            pass
        return out

    bass2jax.compile_bir_kernel = cached_compile


def _get_nc():
    if "nc" not in _CACHE:
        _install_neff_cache()
        _CACHE["nc"] = _build_nc()
    return _CACHE["nc"]


def kernel(input, weight, bias, num_experts_per_token):
    import ml_dtypes
    from concourse.bass_utils import run_bass_kernel_spmd

    bf16 = ml_dtypes.bfloat16
    input = np.ascontiguousarray(np.asarray(input, dtype=np.float32))
    weight = np.ascontiguousarray(np.asarray(weight, dtype=np.float32))
    bias = np.ascontiguousarray(np.asarray(bias, dtype=np.float32))
    counts = np.asarray(num_experts_per_token).astype(np.int64)
    offsets = np.concatenate([[0], np.cumsum(counts)]).astype(np.int64)

    if counts.max() > TOKC:
        # capacity overflow (never hit with balanced routing): numpy fallback
        outs = []
        for i in range(E):
            xi = input[offsets[i]:offsets[i + 1]]
            outs.append(xi @ weight[i].T + bias[i])
        return np.concatenate(outs, axis=0)

    in_maps = []
    for i in range(E):
        xi = input[offsets[i]:offsets[i + 1]]  # [n_i, DIN]
        if xi.shape[0] < TOKC:
            xi = np.concatenate(
                [xi, np.zeros((TOKC - xi.shape[0], DIN), np.float32)], axis=0)
        # [p, m*1024 + k*128 + j] = x[m*128+j, k*128+p]
        xq = np.ascontiguousarray(
            xi.astype(bf16).reshape(MTILES, MT, KTILES, KT)
            .transpose(3, 0, 2, 1).reshape(KT, MTILES * DIN))
        # [p, q*16384 + k*2048 + t*512 + j] = W[(q*4+t)*512+j, k*128+p]
        wq = np.ascontiguousarray(
            weight[i].astype(bf16).reshape(PHASES, NPH, NT, KTILES, KT)
            .transpose(4, 0, 3, 1, 2).reshape(KT, PHASES * WPH))
        bb = np.ascontiguousarray(
            np.broadcast_to(bias[i][None, :], (MT, DOUT)))
        in_maps.append({"xD": xq, "wD": wq, "bD": bb})

    nc = _get_nc()
    import os
    trace = bool(int(os.environ.get("KERNEL_TRACE", "0")))
    res = run_bass_kernel_spmd(nc, in_maps, core_ids=list(range(NCORES)),
                               trace=trace)
    _CACHE["last_result"] = res

    out = np.empty((T, DOUT), dtype=np.float32)
    pos = 0
    for i in range(E):
        n_i = int(counts[i])
        out[pos:pos + n_i] = res.results[i]["y"][:n_i]
        pos += n_i
    return out


# revision 7
# speedup vs baseline: 1.0799x; 1.0358x over previous
"""MoE grouped-GEMM (8 experts) on 8 Trainium2 NeuronCores.

Problem: input [32768, 1024] routed contiguously to 8 experts (counts in
num_experts_per_token); expert i computes x_i @ W_i.T + b_i with
W [8, 4096, 1024], b [8, 4096]. Output [32768, 4096].

Sharding: expert-parallel, expert i <-> core i. Zero collectives; the host
slices per-expert token blocks, packs x and W into SBUF-tile layout, each
core runs a 4096x1024x4096 GEMM (+bias), host concatenates outputs.

Per-core kernel, bf16 operands (rel err ~2e-3 with fp32 PSUM accumulation;
same 1 cycle/row PE throughput as fp32r but half the HBM/SBUF footprint and
cheap pipelineable weight loads):
  - x [128, 32768] and W [128, 32768] bf16 fully SBUF-resident: each is
    read from HBM exactly once
  - 8 n-phases of one 512-col W block (1 MB) each, so compute starts as
    soon as the first W block lands (~11 us); later W blocks and x chunks
    ride the sync queue statically ordered by first-use time, with >=4 KB
    per-partition DMA segments (small segments collapse packet efficiency)
  - loop m -> k accumulates one PSUM bank per m-tile; DVE fuses bias-add
    into the PSUM drain; outputs stream per [128,512] tile on the scalar
    queue (67 MB fp32), the final tiles flush on the idle sync queue
"""

import sys

if "/opt/trn_rl_repo" not in sys.path:
    sys.path.insert(0, "/opt/trn_rl_repo")

import numpy as np

E, T, DIN, DOUT = 8, 32768, 1024, 4096
NCORES = 8
TOKC = T // NCORES  # tokens per core (capacity)

KT = 128   # contraction tile (SBUF partitions)
MT = 128   # token tile (PSUM partitions)
NT = 512   # dout tile (one fp32 PSUM bank)
KTILES = DIN // KT    # 8
MTILES = TOKC // MT   # 32
NTILES = DOUT // NT   # 8
NB = KTILES * NT      # W columns per n-block: 4096

_CACHE = {}


def _build_nc():
    import concourse.bacc as bacc
    import concourse.tile as tile
    import concourse.mybir as mybir

    nc = bacc.Bacc("TRN2", target_bir_lowering=False, debug=False,
                   num_devices=NCORES)

    # xD[p, m*1024 + k*128 + j] = x[m*128 + j, k*128 + p]   (bf16)
    xD = nc.dram_tensor("xD", [KT, MTILES * DIN], mybir.dt.bfloat16,
                        kind="ExternalInput")
    # wD[p, n*4096 + k*512 + j] = W[n*512 + j, k*128 + p]   (bf16)
    wD = nc.dram_tensor("wD", [KT, NTILES * NB], mybir.dt.bfloat16,
                        kind="ExternalInput")
    bD = nc.dram_tensor("bD", [MT, DOUT], mybir.dt.float32,
                        kind="ExternalInput")
    y = nc.dram_tensor("y", [TOKC, DOUT], mybir.dt.float32,
                       kind="ExternalOutput")

    with tile.TileContext(nc) as tc:
        with (
            tc.tile_pool(name="xpool", bufs=1) as xpool,
            tc.tile_pool(name="wpool", bufs=1) as wpool,
            tc.tile_pool(name="bpool", bufs=1) as bpool,
            tc.tile_pool(name="opool", bufs=8) as opool,
            tc.tile_pool(name="psum", bufs=8, space="PSUM") as psum_pool,
        ):
            xS = xpool.tile([KT, MTILES * DIN], mybir.dt.bfloat16,
                            name="xS", tag="xS")
            wS = wpool.tile([KT, NTILES * NB], mybir.dt.bfloat16,
                            name="wS", tag="wS")
            bS = bpool.tile([MT, DOUT], mybir.dt.float32,
                            name="bS", tag="bS")

            def ldw(n):  # one W n-block, 1 MB, 8 KB/partition segments
                nc.sync.dma_start(wS[:, n * NB:(n + 1) * NB],
                                  wD[:, n * NB:(n + 1) * NB])

            def ldx(c):  # one x chunk = 4 m-tiles, 1 MB
                nc.sync.dma_start(xS[:, c * 4 * DIN:(c + 1) * 4 * DIN],
                                  xD[:, c * 4 * DIN:(c + 1) * 4 * DIN])

            # sync queue: everything ordered by first-use time.
            nc.sync.dma_start(xS[:, 0:DIN], xD[:, 0:DIN])        # x m0 gate
            nc.sync.dma_start(wS[:, 0:NB // 2], wD[:, 0:NB // 2])  # W n0 k0-3
            nc.sync.dma_start(wS[:, NB // 2:NB], wD[:, NB // 2:NB])
            ldw(1)
            ldx(2)
            ldx(3)
            nc.sync.dma_start(bS[:, NT:DOUT], bD[:, NT:DOUT])  # bias n1..7
            ldw(2)
            ldx(4)
            ldx(5)
            ldw(3)
            ldx(6)
            ldx(7)
            for n in range(4, NTILES):
                ldw(n)
            # scalar queue: phase-0 bias + x chunk 1 ahead of the outputs.
            nc.scalar.dma_start(bS[:, 0:NT], bD[:, 0:NT])
            nc.scalar.dma_start(xS[:, 4 * DIN:8 * DIN], xD[:, 4 * DIN:8 * DIN])
            # gpsimd (SWDGE): the three early x m-tile gates.
            for m in range(1, 4):
                nc.gpsimd.dma_start(xS[:, m * DIN:(m + 1) * DIN],
                                    xD[:, m * DIN:(m + 1) * DIN])

            for n in range(NTILES):
                for m in range(MTILES):
                    acc = psum_pool.tile([MT, NT], mybir.dt.float32,
                                         name="acc", tag="acc")
                    for k in range(KTILES):
                        nc.tensor.matmul(
                            acc[:],
                            xS[:, m * DIN + k * KT:m * DIN + (k + 1) * KT],
                            wS[:, n * NB + k * NT:n * NB + (k + 1) * NT],
                            start=(k == 0), stop=(k == KTILES - 1))
                    ot = opool.tile([MT, NT], mybir.dt.float32,
                                    name="ot", tag="ot")
                    nc.vector.tensor_add(
                        ot[:], acc[:], bS[:, n * NT:(n + 1) * NT])
                    # final tiles flush on the (by then idle) sync queue so
                    # the kernel tail isn't gated on the scalar backlog
                    oeng = (nc.sync if (n == NTILES - 1 and m >= MTILES - 2)
                            else nc.scalar)
                    oeng.dma_start(
                        y[m * MT:(m + 1) * MT, n * NT:(n + 1) * NT], ot[:])

    nc.compile()
    return nc


def _install_neff_cache():
    """Disk-cache walrus NEFF compiles keyed on the BIR bytes."""
    if _CACHE.get("neff_cache_installed"):
        return
    _CACHE["neff_cache_installed"] = True
    import hashlib
    import os
    import shutil

    import concourse.bass2jax as bass2jax

    cache_dir = "/root/.neff_bir_cache"
    os.makedirs(cache_dir, exist_ok=True)
    orig = bass2jax.compile_bir_kernel

    def cached_compile(ant_bir_str, tmpdir, neff_name="file.neff", **kw):
        key = hashlib.sha256(
            ant_bir_str if isinstance(ant_bir_str, bytes)
            else ant_bir_str.encode()).hexdigest()
        hit = os.path.join(cache_dir, key + ".neff")
        dst = os.path.join(tmpdir, neff_name)
        if os.path.exists(hit):
            shutil.copyfile(hit, dst)
            return dst
        out = orig(ant_bir_str, tmpdir, neff_name=neff_name, **kw)
        try:
            shutil.copyfile(out, hit)
        except OSError:
            pass
        return out

    bass2jax.compile_bir_kernel = cached_compile


def _get_nc():
    if "nc" not in _CACHE:
        _install_neff_cache()
        _CACHE["nc"] = _build_nc()
    return _CACHE["nc"]


def kernel(input, weight, bias, num_experts_per_token):
    import ml_dtypes
    from concourse.bass_utils import run_bass_kernel_spmd

    bf16 = ml_dtypes.bfloat16
    input = np.ascontiguousarray(np.asarray(input, dtype=np.float32))
    weight = np.ascontiguousarray(np.asarray(weight, dtype=np.float32))
    bias = np.ascontiguousarray(np.asarray(bias, dtype=np.float32))
    counts = np.asarray(num_experts_per_token).astype(np.int64)
    offsets = np.concatenate([[0], np.cumsum(counts)]).astype(np.int64)

    if counts.max() > TOKC:
        # capacity overflow (never hit with balanced routing): numpy fallback
        outs = []
        for i in range(E):
            xi = input[offsets[i]:offsets[i + 1]]
            outs.append(xi @ weight[i].T + bias[i])
        return np.concatenate(outs, axis=0)

    in_maps = []
    for i in range(E):
        xi = input[offsets[i]:offsets[i + 1]]  # [n_i, DIN]
        if xi.shape[0] < TOKC:
            xi = np.concatenate(
                [xi, np.zeros((TOKC - xi.shape[0], DIN), np.float32)], axis=0)
        # [p, m*1024 + k*128 + j] = x[m*128+j, k*128+p]
        xq = np.ascontiguousarray(
            xi.astype(bf16).reshape(MTILES, MT, KTILES, KT)
            .transpose(3, 0, 2, 1).reshape(KT, MTILES * DIN))
        # [p, n*4096 + k*512 + j] = W[n*512+j, k*128+p]
        wq = np.ascontiguousarray(
            weight[i].astype(bf16).reshape(NTILES, NT, KTILES, KT)
            .transpose(3, 0, 2, 1).reshape(KT, NTILES * NB))
        bb = np.ascontiguousarray(
            np.broadcast_to(bias[i][None, :], (MT, DOUT)))
        in_maps.append({"xD": xq, "wD": wq, "bD": bb})

    nc = _get_nc()
    import os
    trace = bool(int(os.environ.get("KERNEL_TRACE", "0")))
    res = run_bass_kernel_spmd(nc, in_maps, core_ids=list(range(NCORES)),
                               trace=trace)
    _CACHE["last_result"] = res

    out = np.empty((T, DOUT), dtype=np.float32)
    pos = 0
    for i in range(E):
        n_i = int(counts[i])
        out[pos:pos + n_i] = res.results[i]["y"][:n_i]
        pos += n_i
    return out


# revision 9
# speedup vs baseline: 1.0939x; 1.0130x over previous
"""MoE grouped-GEMM (8 experts) on 8 Trainium2 NeuronCores.

Problem: input [32768, 1024] routed contiguously to 8 experts (counts in
num_experts_per_token); expert i computes x_i @ W_i.T + b_i with
W [8, 4096, 1024], b [8, 4096]. Output [32768, 4096].

Sharding: expert-parallel, expert i <-> core i. Zero collectives; the host
slices per-expert token blocks, packs x and W into SBUF-tile layout, each
core runs a 4096x1024x4096 GEMM (+bias), host concatenates outputs.

Per-core kernel, bf16 operands (rel err ~2e-3 with fp32 PSUM accumulation;
same 1 cycle/row PE throughput as fp32r but half the HBM/SBUF footprint and
cheap pipelineable weight loads):
  - x [128, 32768] and W [128, 32768] bf16 fully SBUF-resident: each is
    read from HBM exactly once
  - 8 n-phases of one 512-col W block (1 MB) each, so compute starts as
    soon as the first W block lands (~11 us); later W blocks and x chunks
    ride the sync queue statically ordered by first-use time, with >=4 KB
    per-partition DMA segments (small segments collapse packet efficiency)
  - loop m -> k accumulates one PSUM bank per m-tile; DVE fuses bias-add
    into the PSUM drain; outputs stream per [128,512] tile on the scalar
    queue (67 MB fp32), the final tiles flush on the idle sync queue
"""

import sys

if "/opt/trn_rl_repo" not in sys.path:
    sys.path.insert(0, "/opt/trn_rl_repo")

import numpy as np

E, T, DIN, DOUT = 8, 32768, 1024, 4096
NCORES = 8
TOKC = T // NCORES  # tokens per core (capacity)

KT = 128   # contraction tile (SBUF partitions)
MT = 128   # token tile (PSUM partitions)
NT = 512   # dout tile (one fp32 PSUM bank)
KTILES = DIN // KT    # 8
MTILES = TOKC // MT   # 32
NTILES = DOUT // NT   # 8
NB = KTILES * NT      # W columns per n-block: 4096

_CACHE = {}


def _build_nc():
    import concourse.bacc as bacc
    import concourse.tile as tile
    import concourse.mybir as mybir

    nc = bacc.Bacc("TRN2", target_bir_lowering=False, debug=False,
                   num_devices=NCORES)

    # xD[p, m*1024 + k*128 + j] = x[m*128 + j, k*128 + p]   (bf16)
    xD = nc.dram_tensor("xD", [KT, MTILES * DIN], mybir.dt.bfloat16,
                        kind="ExternalInput")
    # wD[p, n*4096 + k*512 + j] = W[n*512 + j, k*128 + p]   (bf16)
    wD = nc.dram_tensor("wD", [KT, NTILES * NB], mybir.dt.bfloat16,
                        kind="ExternalInput")
    bD = nc.dram_tensor("bD", [MT, DOUT], mybir.dt.float32,
                        kind="ExternalInput")
    y = nc.dram_tensor("y", [TOKC, DOUT], mybir.dt.float32,
                       kind="ExternalOutput")

    with tile.TileContext(nc) as tc:
        with (
            tc.tile_pool(name="xpool", bufs=1) as xpool,
            tc.tile_pool(name="wpool", bufs=1) as wpool,
            tc.tile_pool(name="bpool", bufs=1) as bpool,
            tc.tile_pool(name="opool", bufs=8) as opool,
            tc.tile_pool(name="psum", bufs=8, space="PSUM") as psum_pool,
        ):
            xS = xpool.tile([KT, MTILES * DIN], mybir.dt.bfloat16,
                            name="xS", tag="xS")
            wS = wpool.tile([KT, NTILES * NB], mybir.dt.bfloat16,
                            name="wS", tag="wS")
            bS = bpool.tile([MT, DOUT], mybir.dt.float32,
                            name="bS", tag="bS")

            def ldw(n):  # one W n-block, 1 MB, 8 KB/partition segments
                nc.sync.dma_start(wS[:, n * NB:(n + 1) * NB],
                                  wD[:, n * NB:(n + 1) * NB])

            def ldx(c):  # one x chunk = 4 m-tiles, 1 MB
                nc.sync.dma_start(xS[:, c * 4 * DIN:(c + 1) * 4 * DIN],
                                  xD[:, c * 4 * DIN:(c + 1) * 4 * DIN])

            # sync queue: everything ordered by first-use time. All
            # transfers are 1 MB with 8 KB/partition segments — smaller
            # segments collapse per-queue DMA throughput (measured:
            # ~250 GB/s at 8 KB vs ~65 GB/s at 4 KB).
            ldx(0)   # x m0-3 gate
            ldw(0)
            ldw(1)
            for c in range(2, 8):
                ldx(c)
            nc.sync.dma_start(bS[:, NT:DOUT], bD[:, NT:DOUT])  # bias n1..7
            for n in range(2, NTILES):
                ldw(n)
            # scalar queue: phase-0 bias + x m4-7, ahead of the outputs.
            nc.scalar.dma_start(bS[:, 0:NT], bD[:, 0:NT])
            nc.scalar.dma_start(xS[:, 4 * DIN:8 * DIN], xD[:, 4 * DIN:8 * DIN])

            for n in range(NTILES):
                for m in range(MTILES):
                    acc = psum_pool.tile([MT, NT], mybir.dt.float32,
                                         name="acc", tag="acc")
                    for k in range(KTILES):
                        nc.tensor.matmul(
                            acc[:],
                            xS[:, m * DIN + k * KT:m * DIN + (k + 1) * KT],
                            wS[:, n * NB + k * NT:n * NB + (k + 1) * NT],
                            start=(k == 0), stop=(k == KTILES - 1))
                    ot = opool.tile([MT, NT], mybir.dt.float32,
                                    name="ot", tag="ot")
                    nc.vector.tensor_add(
                        ot[:], acc[:], bS[:, n * NT:(n + 1) * NT])
                    nc.scalar.dma_start(
                        y[m * MT:(m + 1) * MT, n * NT:(n + 1) * NT], ot[:])

    nc.compile()
    return nc


def _install_neff_cache():
    """Disk-cache walrus NEFF compiles keyed on the BIR bytes."""
    if _CACHE.get("neff_cache_installed"):
        return
    _CACHE["neff_cache_installed"] = True
    import hashlib
    import os
    import shutil

    import concourse.bass2jax as bass2jax

    cache_dir = "/root/.neff_bir_cache"
    os.makedirs(cache_dir, exist_ok=True)
    orig = bass2jax.compile_bir_kernel

    def cached_compile(ant_bir_str, tmpdir, neff_name="file.neff", **kw):
        key = hashlib.sha256(
            ant_bir_str if isinstance(ant_bir_str, bytes)
            else ant_bir_str.encode()).hexdigest()
        hit = os.path.join(cache_dir, key + ".neff")
        dst = os.path.join(tmpdir, neff_name)
        if os.path.exists(hit):
            shutil.copyfile(hit, dst)
            return dst
        out = orig(ant_bir_str, tmpdir, neff_name=neff_name, **kw)
        try:
            shutil.copyfile(out, hit)
        except OSError:
            pass
        return out

    bass2jax.compile_bir_kernel = cached_compile


def _get_nc():
    if "nc" not in _CACHE:
        _install_neff_cache()
        _CACHE["nc"] = _build_nc()
    return _CACHE["nc"]


def kernel(input, weight, bias, num_experts_per_token):
    import ml_dtypes
    from concourse.bass_utils import run_bass_kernel_spmd

    bf16 = ml_dtypes.bfloat16
    input = np.ascontiguousarray(np.asarray(input, dtype=np.float32))
    weight = np.ascontiguousarray(np.asarray(weight, dtype=np.float32))
    bias = np.ascontiguousarray(np.asarray(bias, dtype=np.float32))
    counts = np.asarray(num_experts_per_token).astype(np.int64)
    offsets = np.concatenate([[0], np.cumsum(counts)]).astype(np.int64)

    if counts.max() > TOKC:
        # capacity overflow (never hit with balanced routing): numpy fallback
        outs = []
        for i in range(E):
            xi = input[offsets[i]:offsets[i + 1]]
            outs.append(xi @ weight[i].T + bias[i])
        return np.concatenate(outs, axis=0)

    in_maps = []
    for i in range(E):
        xi = input[offsets[i]:offsets[i + 1]]  # [n_i, DIN]
        if xi.shape[0] < TOKC:
            xi = np.concatenate(
                [xi, np.zeros((TOKC - xi.shape[0], DIN), np.float32)], axis=0)
        # [p, m*1024 + k*128 + j] = x[m*128+j, k*128+p]
        xq = np.ascontiguousarray(
            xi.astype(bf16).reshape(MTILES, MT, KTILES, KT)
            .transpose(3, 0, 2, 1).reshape(KT, MTILES * DIN))
        # [p, n*4096 + k*512 + j] = W[n*512+j, k*128+p]
        wq = np.ascontiguousarray(
            weight[i].astype(bf16).reshape(NTILES, NT, KTILES, KT)
            .transpose(3, 0, 2, 1).reshape(KT, NTILES * NB))
        bb = np.ascontiguousarray(
            np.broadcast_to(bias[i][None, :], (MT, DOUT)))
        in_maps.append({"xD": xq, "wD": wq, "bD": bb})

    nc = _get_nc()
    import os
    trace = bool(int(os.environ.get("KERNEL_TRACE", "0")))
    res = run_bass_kernel_spmd(nc, in_maps, core_ids=list(range(NCORES)),
                               trace=trace)
    _CACHE["last_result"] = res

    out = np.empty((T, DOUT), dtype=np.float32)
    pos = 0
    for i in range(E):
        n_i = int(counts[i])
        out[pos:pos + n_i] = res.results[i]["y"][:n_i]
        pos += n_i
    return out


# revision 10
# speedup vs baseline: 1.2038x; 1.1004x over previous
"""MoE grouped-GEMM (8 experts) on 8 Trainium2 NeuronCores.

Problem: input [32768, 1024] routed contiguously to 8 experts (counts in
num_experts_per_token); expert i computes x_i @ W_i.T + b_i with
W [8, 4096, 1024], b [8, 4096]. Output [32768, 4096].

Sharding: expert-parallel, expert i <-> core i. Zero collectives; the host
slices per-expert token blocks, packs x and W into SBUF-tile layout, each
core runs a 4096x1024x4096 GEMM (+bias), host concatenates outputs.

Per-core kernel (tolerance budget 2e-2 lets precision buy speed):
  - k-tiles 0-5 (DIN 0:768) in bf16 at 1 PE cycle/row; k-tiles 6-7
    (DIN 768:1024) as one fp8-e4m3 DoubleRow matmul at 2x rate
    (x scaled 1/8, W scaled 8 so products land unscaled in the same fp32
    PSUM accumulation). Measured rel err ~1.6e-2 (gate 2e-2).
  - all operands SBUF-resident, each read from HBM exactly once
  - 8 n-phases of one 512-col W block; freight rides the two HWDGE
    queues statically ordered by first-use time with >=6 KB/partition
    segments (smaller segments collapse per-queue DMA throughput)
  - DVE fuses bias-add into the PSUM drain and casts outputs to bf16,
    halving the 67 MB output stream; the host casts back to fp32
"""

import sys

if "/opt/trn_rl_repo" not in sys.path:
    sys.path.insert(0, "/opt/trn_rl_repo")

import os

import numpy as np

E, T, DIN, DOUT = 8, 32768, 1024, 4096
NCORES = 8
TOKC = T // NCORES  # tokens per core (capacity)

KT = 128   # contraction tile (SBUF partitions)
MT = 128   # token tile (PSUM partitions)
NT = 512   # dout tile (one fp32 PSUM bank)
KTILES = DIN // KT    # 8
MTILES = TOKC // MT   # 32
NTILES = DOUT // NT   # 8

FP8 = bool(int(os.environ.get("KERNEL_FP8", "1")))
OUT_BF16 = bool(int(os.environ.get("KERNEL_OUT_BF16", "1")))
KB16 = 6 if FP8 else KTILES   # k-tiles carried in bf16
KF8 = KTILES - KB16           # k-tiles carried in fp8 (DoubleRow pair)
XSCALE = 0.125                # fp8 x scale (W gets 1/XSCALE)
XW = KB16 * KT                # bf16 x columns per m-tile: 768
WW = KB16 * NT                # bf16 W columns per n-block: 3072

_CACHE = {}


def _build_nc():
    import concourse.bacc as bacc
    import concourse.tile as tile
    import concourse.mybir as mybir

    nc = bacc.Bacc("TRN2", target_bir_lowering=False, debug=False,
                   num_devices=NCORES)

    f32 = mybir.dt.float32
    bf16 = mybir.dt.bfloat16
    f8 = mybir.dt.float8e4
    ydt = bf16 if OUT_BF16 else f32

    # xB[p, m*XW + k*128 + j] = x[m*128 + j, k*128 + p], k < KB16
    xBd = nc.dram_tensor("xB", [KT, MTILES * XW], bf16, kind="ExternalInput")
    # wB[p, n*WW + k*512 + j] = W[n*512 + j, k*128 + p], k < KB16
    wBd = nc.dram_tensor("wB", [KT, NTILES * WW], bf16, kind="ExternalInput")
    if FP8:
        # xF[p, r, m*128 + j] = x[m*128+j, KB16*128 + r*128 + p] * XSCALE
        xFd = nc.dram_tensor("xF", [KT, KF8, MTILES * MT], f8,
                             kind="ExternalInput")
        # wF[p, r, n*512 + j] = W[n*512+j, KB16*128 + r*128 + p] / XSCALE
        wFd = nc.dram_tensor("wF", [KT, KF8, NTILES * NT], f8,
                             kind="ExternalInput")
    bD = nc.dram_tensor("bD", [MT, DOUT], f32, kind="ExternalInput")
    y = nc.dram_tensor("y", [TOKC, DOUT], ydt, kind="ExternalOutput")

    with tile.TileContext(nc) as tc:
        with (
            tc.tile_pool(name="xpool", bufs=1) as xpool,
            tc.tile_pool(name="bpool", bufs=1) as bpool,
            tc.tile_pool(name="opool", bufs=8) as opool,
            tc.tile_pool(name="psum", bufs=8, space="PSUM") as psum_pool,
        ):
            xB = xpool.tile([KT, MTILES * XW], bf16, name="xB", tag="xB")
            wB = xpool.tile([KT, NTILES * WW], bf16, name="wB", tag="wB")
            if FP8:
                xF = xpool.tile([KT, KF8, MTILES * MT], f8,
                                name="xF", tag="xF")
                wF = xpool.tile([KT, KF8, NTILES * NT], f8,
                                name="wF", tag="wF")
            bS = bpool.tile([MT, DOUT], f32, name="bS", tag="bS")

            # --- freight, ordered by first-use time -----------------------
            # scalar: phase-0 bias, fp8 W, bf16 W n0, then the outputs.
            nc.scalar.dma_start(bS[:, 0:NT], bD[:, 0:NT])
            if FP8:
                nc.scalar.dma_start(wF[:], wFd[:])
            nc.scalar.dma_start(wB[:, 0:WW], wBd[:, 0:WW])
            # sync: fp8 x, bf16 x m0-3, W n1, x m4..31, bias rest, W n2..7.
            if FP8:
                nc.sync.dma_start(xF[:], xFd[:])
            nc.sync.dma_start(xB[:, 0:4 * XW], xBd[:, 0:4 * XW])
            nc.sync.dma_start(wB[:, WW:2 * WW], wBd[:, WW:2 * WW])
            for c in range(1, 8):
                nc.sync.dma_start(xB[:, c * 4 * XW:(c + 1) * 4 * XW],
                                  xBd[:, c * 4 * XW:(c + 1) * 4 * XW])
            nc.sync.dma_start(bS[:, NT:DOUT], bD[:, NT:DOUT])
            for n in range(2, NTILES):
                nc.sync.dma_start(wB[:, n * WW:(n + 1) * WW],
                                  wBd[:, n * WW:(n + 1) * WW])

            for n in range(NTILES):
                for m in range(MTILES):
                    acc = psum_pool.tile([MT, NT], mybir.dt.float32,
                                         name="acc", tag="acc")
                    for k in range(KB16):
                        nc.tensor.matmul(
                            acc[:],
                            xB[:, m * XW + k * KT:m * XW + (k + 1) * KT],
                            wB[:, n * WW + k * NT:n * WW + (k + 1) * NT],
                            start=(k == 0), stop=(not FP8 and k == KB16 - 1))
                    if FP8:
                        nc.tensor.matmul(
                            acc[:],
                            xF[:, :, m * MT:(m + 1) * MT],
                            wF[:, :, n * NT:(n + 1) * NT],
                            start=False, stop=True,
                            perf_mode=mybir.MatmulPerfMode.DoubleRow)
                    ot = opool.tile([MT, NT], ydt, name="ot", tag="ot")
                    nc.vector.tensor_add(
                        ot[:], acc[:], bS[:, n * NT:(n + 1) * NT])
                    nc.scalar.dma_start(
                        y[m * MT:(m + 1) * MT, n * NT:(n + 1) * NT], ot[:])

    nc.compile()
    return nc


def _install_neff_cache():
    """Disk-cache walrus NEFF compiles keyed on the BIR bytes."""
    if _CACHE.get("neff_cache_installed"):
        return
    _CACHE["neff_cache_installed"] = True
    import hashlib
    import shutil

    import concourse.bass2jax as bass2jax

    cache_dir = "/root/.neff_bir_cache"
    os.makedirs(cache_dir, exist_ok=True)
    orig = bass2jax.compile_bir_kernel

    def cached_compile(ant_bir_str, tmpdir, neff_name="file.neff", **kw):
        key = hashlib.sha256(
            ant_bir_str if isinstance(ant_bir_str, bytes)
            else ant_bir_str.encode()).hexdigest()
        hit = os.path.join(cache_dir, key + ".neff")
        dst = os.path.join(tmpdir, neff_name)
        if os.path.exists(hit):
            shutil.copyfile(hit, dst)
            return dst
        out = orig(ant_bir_str, tmpdir, neff_name=neff_name, **kw)
        try:
            shutil.copyfile(out, hit)
        except OSError:
            pass
        return out

    bass2jax.compile_bir_kernel = cached_compile


def _get_nc():
    if "nc" not in _CACHE:
        _install_neff_cache()
        _CACHE["nc"] = _build_nc()
    return _CACHE["nc"]


def kernel(input, weight, bias, num_experts_per_token):
    import ml_dtypes
    from concourse.bass_utils import run_bass_kernel_spmd

    bf16 = ml_dtypes.bfloat16
    f8 = ml_dtypes.float8_e4m3
    input = np.ascontiguousarray(np.asarray(input, dtype=np.float32))
    weight = np.ascontiguousarray(np.asarray(weight, dtype=np.float32))
    bias = np.ascontiguousarray(np.asarray(bias, dtype=np.float32))
    counts = np.asarray(num_experts_per_token).astype(np.int64)
    offsets = np.concatenate([[0], np.cumsum(counts)]).astype(np.int64)

    if counts.max() > TOKC:
        # capacity overflow (never hit with balanced routing): numpy fallback
        outs = []
        for i in range(E):
            xi = input[offsets[i]:offsets[i + 1]]
            outs.append(xi @ weight[i].T + bias[i])
        return np.concatenate(outs, axis=0)

    KB = KB16 * KT  # bf16 DIN columns
    in_maps = []
    for i in range(E):
        xi = input[offsets[i]:offsets[i + 1]]  # [n_i, DIN]
        if xi.shape[0] < TOKC:
            xi = np.concatenate(
                [xi, np.zeros((TOKC - xi.shape[0], DIN), np.float32)], axis=0)
        wi = weight[i]
        m = {
            "xB": np.ascontiguousarray(
                xi[:, :KB].astype(bf16).reshape(MTILES, MT, KB16, KT)
                .transpose(3, 0, 2, 1).reshape(KT, MTILES * XW)),
            "wB": np.ascontiguousarray(
                wi[:, :KB].astype(bf16).reshape(NTILES, NT, KB16, KT)
                .transpose(3, 0, 2, 1).reshape(KT, NTILES * WW)),
            "bD": np.ascontiguousarray(
                np.broadcast_to(bias[i][None, :], (MT, DOUT))),
        }
        if FP8:
            m["xF"] = np.ascontiguousarray(
                (xi[:, KB:] * XSCALE).astype(f8)
                .reshape(MTILES, MT, KF8, KT)
                .transpose(3, 2, 0, 1).reshape(KT, KF8, MTILES * MT))
            m["wF"] = np.ascontiguousarray(
                (wi[:, KB:] * (1.0 / XSCALE)).astype(f8)
                .reshape(NTILES, NT, KF8, KT)
                .transpose(3, 2, 0, 1).reshape(KT, KF8, NTILES * NT))
        in_maps.append(m)

    nc = _get_nc()
    trace = bool(int(os.environ.get("KERNEL_TRACE", "0")))
    res = run_bass_kernel_spmd(nc, in_maps, core_ids=list(range(NCORES)),
                               trace=trace)
    _CACHE["last_result"] = res

    out = np.empty((T, DOUT), dtype=np.float32)
    pos = 0
    for i in range(E):
        n_i = int(counts[i])
        out[pos:pos + n_i] = res.results[i]["y"][:n_i].astype(np.float32)
        pos += n_i
    return out


# revision 19
# speedup vs baseline: 1.2210x; 1.0143x over previous
"""MoE grouped-GEMM (8 experts) on 8 Trainium2 NeuronCores.

Problem: input [32768, 1024] routed contiguously to 8 experts (counts in
num_experts_per_token); expert i computes x_i @ W_i.T + b_i with
W [8, 4096, 1024], b [8, 4096]. Output [32768, 4096].

Sharding: expert-parallel, expert i <-> core i. Zero collectives; the host
slices per-expert token blocks, packs x and W into SBUF-tile layout, each
core runs a 4096x1024x4096 GEMM (+bias), host concatenates outputs.

Per-core kernel (tolerance budget 2e-2 lets precision buy speed):
  - k-tiles 0-5 (DIN 0:768) in bf16 at 1 PE cycle/row; k-tiles 6-7
    (DIN 768:1024) as one fp8-e4m3 DoubleRow matmul at 2x rate
    (x scaled down, W scaled up by the same factor so products land
    unscaled in the same fp32 PSUM accumulation). Measured rel err
    1.89e-2 (gate 2e-2), bit-exact reproducible across runs.
  - all operands SBUF-resident, each read from HBM exactly once
  - 8 n-phases of one 512-col W block; freight rides the two HWDGE
    queues statically ordered by first-use time with >=6 KB/partition
    segments (smaller segments collapse per-queue DMA throughput)
  - DVE fuses bias-add into the PSUM drain and casts outputs to bf16,
    halving the 67 MB output stream; the host casts back to fp32
"""

import sys

if "/opt/trn_rl_repo" not in sys.path:
    sys.path.insert(0, "/opt/trn_rl_repo")

import os

import numpy as np

E, T, DIN, DOUT = 8, 32768, 1024, 4096
NCORES = 8
TOKC = T // NCORES  # tokens per core (capacity)

KT = 128   # contraction tile (SBUF partitions)
MT = 128   # token tile (PSUM partitions)
NT = 512   # dout tile (one fp32 PSUM bank)
KTILES = DIN // KT    # 8
MTILES = TOKC // MT   # 32
NTILES = DOUT // NT   # 8

FP8 = bool(int(os.environ.get("KERNEL_FP8", "1")))
OUT_BF16 = bool(int(os.environ.get("KERNEL_OUT_BF16", "1")))
KB16 = 6 if FP8 else KTILES   # k-tiles carried in bf16
KF8 = KTILES - KB16           # k-tiles carried in fp8 (DoubleRow pair)
XSCALE = 0.1768               # fp8 x scale (W gets 1/XSCALE); hw-scanned
                              # minimum of rel err (flat 1.891-1.897e-2
                              # across 0.088-0.25)
XW = KB16 * KT                # bf16 x columns per m-tile: 768
WW = KB16 * NT                # bf16 W columns per n-block: 3072

_CACHE = {}


def _build_nc():
    import concourse.bacc as bacc
    import concourse.tile as tile
    import concourse.mybir as mybir

    nc = bacc.Bacc("TRN2", target_bir_lowering=False, debug=False,
                   num_devices=NCORES)

    f32 = mybir.dt.float32
    bf16 = mybir.dt.bfloat16
    f8 = mybir.dt.float8e4
    ydt = bf16 if OUT_BF16 else f32

    # xB[p, m*XW + k*128 + j] = x[m*128 + j, k*128 + p], k < KB16
    xBd = nc.dram_tensor("xB", [KT, MTILES * XW], bf16, kind="ExternalInput")
    # wB[p, n*WW + k*512 + j] = W[n*512 + j, k*128 + p], k < KB16
    wBd = nc.dram_tensor("wB", [KT, NTILES * WW], bf16, kind="ExternalInput")
    if FP8:
        # xF[p, r, m*128 + j] = x[m*128+j, KB16*128 + r*128 + p] * XSCALE
        xFd = nc.dram_tensor("xF", [KT, KF8, MTILES * MT], f8,
                             kind="ExternalInput")
        # wF[p, r, n*512 + j] = W[n*512+j, KB16*128 + r*128 + p] / XSCALE
        wFd = nc.dram_tensor("wF", [KT, KF8, NTILES * NT], f8,
                             kind="ExternalInput")
    bD = nc.dram_tensor("bD", [MT, DOUT], f32, kind="ExternalInput")
    y = nc.dram_tensor("y", [TOKC, DOUT], ydt, kind="ExternalOutput")

    with tile.TileContext(nc) as tc:
        with (
            tc.tile_pool(name="xpool", bufs=1) as xpool,
            tc.tile_pool(name="bpool", bufs=1) as bpool,
            tc.tile_pool(name="opool", bufs=8) as opool,
            tc.tile_pool(name="psum", bufs=8, space="PSUM") as psum_pool,
        ):
            xB = xpool.tile([KT, MTILES * XW], bf16, name="xB", tag="xB")
            wB = xpool.tile([KT, NTILES * WW], bf16, name="wB", tag="wB")
            if FP8:
                xF = xpool.tile([KT, KF8, MTILES * MT], f8,
                                name="xF", tag="xF")
                wF = xpool.tile([KT, KF8, NTILES * NT], f8,
                                name="wF", tag="wF")
            bS = bpool.tile([MT, DOUT], f32, name="bS", tag="bS")

            # --- freight, ordered by first-use time -----------------------
            # scalar: bf16 W n0, fp8 W, phase-0 bias, then the outputs.
            nc.scalar.dma_start(wB[:, 0:WW], wBd[:, 0:WW])
            if FP8:
                nc.scalar.dma_start(wF[:], wFd[:])
            nc.scalar.dma_start(bS[:, 0:NT], bD[:, 0:NT])
            # sync: bf16 x m0-3, fp8 x, x m4-11, W n1, x m12..31, bias
            # rest, W n2..7.
            nc.sync.dma_start(xB[:, 0:4 * XW], xBd[:, 0:4 * XW])
            if FP8:
                nc.sync.dma_start(xF[:], xFd[:])
            for c in (1, 2):
                nc.sync.dma_start(xB[:, c * 4 * XW:(c + 1) * 4 * XW],
                                  xBd[:, c * 4 * XW:(c + 1) * 4 * XW])
            nc.sync.dma_start(wB[:, WW:2 * WW], wBd[:, WW:2 * WW])
            for c in range(3, 8):
                nc.sync.dma_start(xB[:, c * 4 * XW:(c + 1) * 4 * XW],
                                  xBd[:, c * 4 * XW:(c + 1) * 4 * XW])
            nc.sync.dma_start(bS[:, NT:DOUT], bD[:, NT:DOUT])
            for n in range(2, NTILES):
                nc.sync.dma_start(wB[:, n * WW:(n + 1) * WW],
                                  wBd[:, n * WW:(n + 1) * WW])

            for n in range(NTILES):
                for m in range(MTILES):
                    acc = psum_pool.tile([MT, NT], mybir.dt.float32,
                                         name="acc", tag="acc")
                    for k in range(KB16):
                        nc.tensor.matmul(
                            acc[:],
                            xB[:, m * XW + k * KT:m * XW + (k + 1) * KT],
                            wB[:, n * WW + k * NT:n * WW + (k + 1) * NT],
                            start=(k == 0), stop=(not FP8 and k == KB16 - 1))
                    if FP8:
                        # DoubleRow accumulate-last: start=True on a DR
                        # matmul measurably corrupts the accumulation
                        nc.tensor.matmul(
                            acc[:],
                            xF[:, :, m * MT:(m + 1) * MT],
                            wF[:, :, n * NT:(n + 1) * NT],
                            start=False, stop=True,
                            perf_mode=mybir.MatmulPerfMode.DoubleRow)
                    ot = opool.tile([MT, NT], ydt, name="ot", tag="ot")
                    nc.vector.tensor_add(
                        ot[:], acc[:], bS[:, n * NT:(n + 1) * NT])
                    nc.scalar.dma_start(
                        y[m * MT:(m + 1) * MT, n * NT:(n + 1) * NT], ot[:])

    nc.compile()
    return nc


def _install_neff_cache():
    """Disk-cache walrus NEFF compiles keyed on the BIR bytes."""
    if _CACHE.get("neff_cache_installed"):
        return
    _CACHE["neff_cache_installed"] = True
    import hashlib
    import shutil

    import concourse.bass2jax as bass2jax

    cache_dir = "/root/.neff_bir_cache"
    os.makedirs(cache_dir, exist_ok=True)
    orig = bass2jax.compile_bir_kernel

    def cached_compile(ant_bir_str, tmpdir, neff_name="file.neff", **kw):
        key = hashlib.sha256(
            ant_bir_str if isinstance(ant_bir_str, bytes)
            else ant_bir_str.encode()).hexdigest()
        hit = os.path.join(cache_dir, key + ".neff")
        dst = os.path.join(tmpdir, neff_name)
        if os.path.exists(hit):
            shutil.copyfile(hit, dst)
            return dst
        out = orig(ant_bir_str, tmpdir, neff_name=neff_name, **kw)
        try:
            shutil.copyfile(out, hit)
        except OSError:
            pass
        return out

    bass2jax.compile_bir_kernel = cached_compile


def _get_nc():
    if "nc" not in _CACHE:
        _install_neff_cache()
        _CACHE["nc"] = _build_nc()
    return _CACHE["nc"]


def kernel(input, weight, bias, num_experts_per_token):
    import ml_dtypes
    from concourse.bass_utils import run_bass_kernel_spmd

    bf16 = ml_dtypes.bfloat16
    f8 = ml_dtypes.float8_e4m3
    input = np.ascontiguousarray(np.asarray(input, dtype=np.float32))
    weight = np.ascontiguousarray(np.asarray(weight, dtype=np.float32))
    bias = np.ascontiguousarray(np.asarray(bias, dtype=np.float32))
    counts = np.asarray(num_experts_per_token).astype(np.int64)
    offsets = np.concatenate([[0], np.cumsum(counts)]).astype(np.int64)

    if counts.max() > TOKC:
        # capacity overflow (never hit with balanced routing): numpy fallback
        outs = []
        for i in range(E):
            xi = input[offsets[i]:offsets[i + 1]]
            outs.append(xi @ weight[i].T + bias[i])
        return np.concatenate(outs, axis=0)

    KB = KB16 * KT  # bf16 DIN columns
    MIN_NORMAL = 2.0 ** -6

    def q8(a):
        """Plain e4m3 RTN. The PE's DoubleRow path reads values at and
        below the min-normal as 0 (measured); promoting them to nearby
        normals makes hardware error worse, so leave them be."""
        return a.astype(f8)

    in_maps = []
    for i in range(E):
        xi = input[offsets[i]:offsets[i + 1]]  # [n_i, DIN]
        if xi.shape[0] < TOKC:
            xi = np.concatenate(
                [xi, np.zeros((TOKC - xi.shape[0], DIN), np.float32)], axis=0)
        wi = weight[i]
        m = {
            "xB": np.ascontiguousarray(
                xi[:, :KB].astype(bf16).reshape(MTILES, MT, KB16, KT)
                .transpose(3, 0, 2, 1).reshape(KT, MTILES * XW)),
            "wB": np.ascontiguousarray(
                wi[:, :KB].astype(bf16).reshape(NTILES, NT, KB16, KT)
                .transpose(3, 0, 2, 1).reshape(KT, NTILES * WW)),
            "bD": np.ascontiguousarray(
                np.broadcast_to(bias[i][None, :], (MT, DOUT))),
        }
        if FP8:
            m["xF"] = np.ascontiguousarray(
                q8(xi[:, KB:] * XSCALE)
                .reshape(MTILES, MT, KF8, KT)
                .transpose(3, 2, 0, 1).reshape(KT, KF8, MTILES * MT))
            m["wF"] = np.ascontiguousarray(
                q8(wi[:, KB:] * (1.0 / XSCALE))
                .reshape(NTILES, NT, KF8, KT)
                .transpose(3, 2, 0, 1).reshape(KT, KF8, NTILES * NT))
        in_maps.append(m)

    nc = _get_nc()
    trace = bool(int(os.environ.get("KERNEL_TRACE", "0")))
    res = run_bass_kernel_spmd(nc, in_maps, core_ids=list(range(NCORES)),
                               trace=trace)
    _CACHE["last_result"] = res

    out = np.empty((T, DOUT), dtype=np.float32)
    pos = 0
    for i in range(E):
        n_i = int(counts[i])
        out[pos:pos + n_i] = res.results[i]["y"][:n_i].astype(np.float32)
        pos += n_i
    return out


# revision 21
# speedup vs baseline: 1.2236x; 1.0022x over previous
"""MoE grouped-GEMM (8 experts) on 8 Trainium2 NeuronCores.

Problem: input [32768, 1024] routed contiguously to 8 experts (counts in
num_experts_per_token); expert i computes x_i @ W_i.T + b_i with
W [8, 4096, 1024], b [8, 4096]. Output [32768, 4096].

Sharding: expert-parallel, expert i <-> core i. Zero collectives; the host
slices per-expert token blocks, packs x and W into SBUF-tile layout, each
core runs a 4096x1024x4096 GEMM (+bias), host concatenates outputs.

Per-core kernel (tolerance budget 2e-2 lets precision buy speed):
  - k-tiles 0-5 (DIN 0:768) in bf16 at 1 PE cycle/row; k-tiles 6-7
    (DIN 768:1024) as one fp8-e4m3 DoubleRow matmul at 2x rate
    (x scaled down, W scaled up by the same factor so products land
    unscaled in the same fp32 PSUM accumulation). Measured rel err
    1.89e-2 (gate 2e-2), bit-exact reproducible across runs.
  - all operands SBUF-resident, each read from HBM exactly once
  - 8 n-phases of one 512-col W block; freight rides the two HWDGE
    queues statically ordered by first-use time with >=6 KB/partition
    segments (smaller segments collapse per-queue DMA throughput)
  - DVE fuses bias-add into the PSUM drain and casts outputs to bf16,
    halving the 67 MB output stream; the host casts back to fp32
"""

import sys

if "/opt/trn_rl_repo" not in sys.path:
    sys.path.insert(0, "/opt/trn_rl_repo")

import os

import numpy as np

E, T, DIN, DOUT = 8, 32768, 1024, 4096
NCORES = 8
TOKC = T // NCORES  # tokens per core (capacity)

KT = 128   # contraction tile (SBUF partitions)
MT = 128   # token tile (PSUM partitions)
NT = 512   # dout tile (one fp32 PSUM bank)
KTILES = DIN // KT    # 8
MTILES = TOKC // MT   # 32
NTILES = DOUT // NT   # 8

FP8 = bool(int(os.environ.get("KERNEL_FP8", "1")))
OUT_BF16 = bool(int(os.environ.get("KERNEL_OUT_BF16", "1")))
KB16 = 6 if FP8 else KTILES   # k-tiles carried in bf16
KF8 = KTILES - KB16           # k-tiles carried in fp8 (DoubleRow pair)
XSCALE = 0.1768               # fp8 x scale (W gets 1/XSCALE); hw-scanned
                              # minimum of rel err (flat 1.891-1.897e-2
                              # across 0.088-0.25)
XW = KB16 * KT                # bf16 x columns per m-tile: 768
WW = KB16 * NT                # bf16 W columns per n-block: 3072

_CACHE = {}


def _build_nc():
    import concourse.bacc as bacc
    import concourse.tile as tile
    import concourse.mybir as mybir

    nc = bacc.Bacc("TRN2", target_bir_lowering=False, debug=False,
                   num_devices=NCORES)

    f32 = mybir.dt.float32
    bf16 = mybir.dt.bfloat16
    f8 = mybir.dt.float8e4
    ydt = bf16 if OUT_BF16 else f32

    # xB[p, m*XW + k*128 + j] = x[m*128 + j, k*128 + p], k < KB16
    xBd = nc.dram_tensor("xB", [KT, MTILES * XW], bf16, kind="ExternalInput")
    # wB[p, n*WW + k*512 + j] = W[n*512 + j, k*128 + p], k < KB16
    wBd = nc.dram_tensor("wB", [KT, NTILES * WW], bf16, kind="ExternalInput")
    if FP8:
        # xF[p, r, m*128 + j] = x[m*128+j, KB16*128 + r*128 + p] * XSCALE
        xFd = nc.dram_tensor("xF", [KT, KF8, MTILES * MT], f8,
                             kind="ExternalInput")
        # wF[p, r, n*512 + j] = W[n*512+j, KB16*128 + r*128 + p] / XSCALE
        wFd = nc.dram_tensor("wF", [KT, KF8, NTILES * NT], f8,
                             kind="ExternalInput")
    bD = nc.dram_tensor("bD", [MT, DOUT], f32, kind="ExternalInput")
    y = nc.dram_tensor("y", [TOKC, DOUT], ydt, kind="ExternalOutput")

    with tile.TileContext(nc) as tc:
        with (
            tc.tile_pool(name="xpool", bufs=1) as xpool,
            tc.tile_pool(name="bpool", bufs=1) as bpool,
            tc.tile_pool(name="opool", bufs=8) as opool,
            tc.tile_pool(name="psum", bufs=8, space="PSUM") as psum_pool,
        ):
            xB = xpool.tile([KT, MTILES * XW], bf16, name="xB", tag="xB")
            wB = xpool.tile([KT, NTILES * WW], bf16, name="wB", tag="wB")
            if FP8:
                xF = xpool.tile([KT, KF8, MTILES * MT], f8,
                                name="xF", tag="xF")
                wF = xpool.tile([KT, KF8, NTILES * NT], f8,
                                name="wF", tag="wF")
            bS = bpool.tile([MT, DOUT], f32, name="bS", tag="bS")

            # --- freight, ordered by first-use time -----------------------
            # The sync (SP) queue moves ~2x the early-window bandwidth of
            # the scalar (Activation) queue, so everything group-0 needs
            # rides sync; fp8 tensors split in halves so the first DR
            # matmul gates on 0.5 MB, not 1 MB.
            nc.sync.dma_start(wB[:, 0:WW], wBd[:, 0:WW])          # W n0
            if FP8:
                HF = NTILES * NT // 2
                nc.sync.dma_start(wF[:, :, 0:HF], wFd[:, :, 0:HF])  # n0-3
                HX = MTILES * MT // 2
                nc.sync.dma_start(xF[:, :, 0:HX], xFd[:, :, 0:HX])  # m0-15
            for c in (1, 2):
                nc.sync.dma_start(xB[:, c * 4 * XW:(c + 1) * 4 * XW],
                                  xBd[:, c * 4 * XW:(c + 1) * 4 * XW])
            nc.sync.dma_start(wB[:, WW:2 * WW], wBd[:, WW:2 * WW])
            for c in range(3, 8):
                nc.sync.dma_start(xB[:, c * 4 * XW:(c + 1) * 4 * XW],
                                  xBd[:, c * 4 * XW:(c + 1) * 4 * XW])
            nc.sync.dma_start(bS[:, NT:DOUT], bD[:, NT:DOUT])
            for n in range(2, NTILES):
                nc.sync.dma_start(wB[:, n * WW:(n + 1) * WW],
                                  wBd[:, n * WW:(n + 1) * WW])
            # scalar: bf16 x m0-3, phase-0 bias, late fp8 halves, outputs.
            nc.scalar.dma_start(xB[:, 0:4 * XW], xBd[:, 0:4 * XW])
            nc.scalar.dma_start(bS[:, 0:NT], bD[:, 0:NT])
            if FP8:
                nc.scalar.dma_start(wF[:, :, HF:2 * HF], wFd[:, :, HF:2 * HF])
                nc.scalar.dma_start(xF[:, :, HX:2 * HX], xFd[:, :, HX:2 * HX])

            for n in range(NTILES):
                for m in range(MTILES):
                    acc = psum_pool.tile([MT, NT], mybir.dt.float32,
                                         name="acc", tag="acc")
                    for k in range(KB16):
                        nc.tensor.matmul(
                            acc[:],
                            xB[:, m * XW + k * KT:m * XW + (k + 1) * KT],
                            wB[:, n * WW + k * NT:n * WW + (k + 1) * NT],
                            start=(k == 0), stop=(not FP8 and k == KB16 - 1))
                    if FP8:
                        # DoubleRow accumulate-last: start=True on a DR
                        # matmul measurably corrupts the accumulation
                        nc.tensor.matmul(
                            acc[:],
                            xF[:, :, m * MT:(m + 1) * MT],
                            wF[:, :, n * NT:(n + 1) * NT],
                            start=False, stop=True,
                            perf_mode=mybir.MatmulPerfMode.DoubleRow)
                    ot = opool.tile([MT, NT], ydt, name="ot", tag="ot")
                    nc.vector.tensor_add(
                        ot[:], acc[:], bS[:, n * NT:(n + 1) * NT])
                    # final two tiles flush on the (by then idle) sync
                    # queue so the tail isn't serialized behind scalar
                    oeng = (nc.sync if (n == NTILES - 1 and m >= MTILES - 2)
                            else nc.scalar)
                    oeng.dma_start(
                        y[m * MT:(m + 1) * MT, n * NT:(n + 1) * NT], ot[:])

    nc.compile()
    return nc


def _install_neff_cache():
    """Disk-cache walrus NEFF compiles keyed on the BIR bytes."""
    if _CACHE.get("neff_cache_installed"):
        return
    _CACHE["neff_cache_installed"] = True
    import hashlib
    import shutil

    import concourse.bass2jax as bass2jax

    cache_dir = "/root/.neff_bir_cache"
    os.makedirs(cache_dir, exist_ok=True)
    orig = bass2jax.compile_bir_kernel

    def cached_compile(ant_bir_str, tmpdir, neff_name="file.neff", **kw):
        key = hashlib.sha256(
            ant_bir_str if isinstance(ant_bir_str, bytes)
            else ant_bir_str.encode()).hexdigest()
        hit = os.path.join(cache_dir, key + ".neff")
        dst = os.path.join(tmpdir, neff_name)
        if os.path.exists(hit):
            shutil.copyfile(hit, dst)
            return dst
        out = orig(ant_bir_str, tmpdir, neff_name=neff_name, **kw)
        try:
            shutil.copyfile(out, hit)
        except OSError:
            pass
        return out

    bass2jax.compile_bir_kernel = cached_compile


def _get_nc():
    if "nc" not in _CACHE:
        _install_neff_cache()
        _CACHE["nc"] = _build_nc()
    return _CACHE["nc"]


def kernel(input, weight, bias, num_experts_per_token):
    import ml_dtypes
    from concourse.bass_utils import run_bass_kernel_spmd

    bf16 = ml_dtypes.bfloat16
    f8 = ml_dtypes.float8_e4m3
    input = np.ascontiguousarray(np.asarray(input, dtype=np.float32))
    weight = np.ascontiguousarray(np.asarray(weight, dtype=np.float32))
    bias = np.ascontiguousarray(np.asarray(bias, dtype=np.float32))
    counts = np.asarray(num_experts_per_token).astype(np.int64)
    offsets = np.concatenate([[0], np.cumsum(counts)]).astype(np.int64)

    if counts.max() > TOKC:
        # capacity overflow (never hit with balanced routing): numpy fallback
        outs = []
        for i in range(E):
            xi = input[offsets[i]:offsets[i + 1]]
            outs.append(xi @ weight[i].T + bias[i])
        return np.concatenate(outs, axis=0)

    KB = KB16 * KT  # bf16 DIN columns
    MIN_NORMAL = 2.0 ** -6

    def q8(a):
        """Plain e4m3 RTN. The PE's DoubleRow path reads values at and
        below the min-normal as 0 (measured); promoting them to nearby
        normals makes hardware error worse, so leave them be."""
        return a.astype(f8)

    in_maps = []
    for i in range(E):
        xi = input[offsets[i]:offsets[i + 1]]  # [n_i, DIN]
        if xi.shape[0] < TOKC:
            xi = np.concatenate(
                [xi, np.zeros((TOKC - xi.shape[0], DIN), np.float32)], axis=0)
        wi = weight[i]
        m = {
            "xB": np.ascontiguousarray(
                xi[:, :KB].astype(bf16).reshape(MTILES, MT, KB16, KT)
                .transpose(3, 0, 2, 1).reshape(KT, MTILES * XW)),
            "wB": np.ascontiguousarray(
                wi[:, :KB].astype(bf16).reshape(NTILES, NT, KB16, KT)
                .transpose(3, 0, 2, 1).reshape(KT, NTILES * WW)),
            "bD": np.ascontiguousarray(
                np.broadcast_to(bias[i][None, :], (MT, DOUT))),
        }
        if FP8:
            m["xF"] = np.ascontiguousarray(
                q8(xi[:, KB:] * XSCALE)
                .reshape(MTILES, MT, KF8, KT)
                .transpose(3, 2, 0, 1).reshape(KT, KF8, MTILES * MT))
            m["wF"] = np.ascontiguousarray(
                q8(wi[:, KB:] * (1.0 / XSCALE))
                .reshape(NTILES, NT, KF8, KT)
                .transpose(3, 2, 0, 1).reshape(KT, KF8, NTILES * NT))
        in_maps.append(m)

    nc = _get_nc()
    trace = bool(int(os.environ.get("KERNEL_TRACE", "0")))
    res = run_bass_kernel_spmd(nc, in_maps, core_ids=list(range(NCORES)),
                               trace=trace)
    _CACHE["last_result"] = res

    out = np.empty((T, DOUT), dtype=np.float32)
    pos = 0
    for i in range(E):
        n_i = int(counts[i])
        out[pos:pos + n_i] = res.results[i]["y"][:n_i].astype(np.float32)
        pos += n_i
    return out


# revision 22
# speedup vs baseline: 1.2254x; 1.0014x over previous
"""MoE grouped-GEMM (8 experts) on 8 Trainium2 NeuronCores.

Problem: input [32768, 1024] routed contiguously to 8 experts (counts in
num_experts_per_token); expert i computes x_i @ W_i.T + b_i with
W [8, 4096, 1024], b [8, 4096]. Output [32768, 4096].

Sharding: expert-parallel, expert i <-> core i. Zero collectives; the host
slices per-expert token blocks, packs x and W into SBUF-tile layout, each
core runs a 4096x1024x4096 GEMM (+bias), host concatenates outputs.

Per-core kernel (tolerance budget 2e-2 lets precision buy speed):
  - k-tiles 0-5 (DIN 0:768) in bf16 at 1 PE cycle/row; k-tiles 6-7
    (DIN 768:1024) as one fp8-e4m3 DoubleRow matmul at 2x rate
    (x scaled down, W scaled up by the same factor so products land
    unscaled in the same fp32 PSUM accumulation). Measured rel err
    1.89e-2 (gate 2e-2), bit-exact reproducible across runs.
  - all operands SBUF-resident, each read from HBM exactly once
  - 8 n-phases of one 512-col W block; freight rides the two HWDGE
    queues statically ordered by first-use time with >=6 KB/partition
    segments (smaller segments collapse per-queue DMA throughput)
  - DVE fuses bias-add into the PSUM drain and casts outputs to bf16,
    halving the 67 MB output stream; the host casts back to fp32
"""

import sys

if "/opt/trn_rl_repo" not in sys.path:
    sys.path.insert(0, "/opt/trn_rl_repo")

import os

import numpy as np

E, T, DIN, DOUT = 8, 32768, 1024, 4096
NCORES = 8
TOKC = T // NCORES  # tokens per core (capacity)

KT = 128   # contraction tile (SBUF partitions)
MT = 128   # token tile (PSUM partitions)
NT = 512   # dout tile (one fp32 PSUM bank)
KTILES = DIN // KT    # 8
MTILES = TOKC // MT   # 32
NTILES = DOUT // NT   # 8

FP8 = bool(int(os.environ.get("KERNEL_FP8", "1")))
OUT_BF16 = bool(int(os.environ.get("KERNEL_OUT_BF16", "1")))
KB16 = 6 if FP8 else KTILES   # k-tiles carried in bf16
KF8 = KTILES - KB16           # k-tiles carried in fp8 (DoubleRow pair)
XSCALE = 0.1768               # fp8 x scale (W gets 1/XSCALE); hw-scanned
                              # minimum of rel err (flat 1.891-1.897e-2
                              # across 0.088-0.25)
XW = KB16 * KT                # bf16 x columns per m-tile: 768
WW = KB16 * NT                # bf16 W columns per n-block: 3072

_CACHE = {}


def _build_nc():
    import concourse.bacc as bacc
    import concourse.tile as tile
    import concourse.mybir as mybir

    nc = bacc.Bacc("TRN2", target_bir_lowering=False, debug=False,
                   num_devices=NCORES)

    f32 = mybir.dt.float32
    bf16 = mybir.dt.bfloat16
    f8 = mybir.dt.float8e4
    ydt = bf16 if OUT_BF16 else f32

    # xB[p, m*XW + k*128 + j] = x[m*128 + j, k*128 + p], k < KB16
    xBd = nc.dram_tensor("xB", [KT, MTILES * XW], bf16, kind="ExternalInput")
    # wB[p, n*WW + k*512 + j] = W[n*512 + j, k*128 + p], k < KB16
    wBd = nc.dram_tensor("wB", [KT, NTILES * WW], bf16, kind="ExternalInput")
    if FP8:
        # xF[p, r, m*128 + j] = x[m*128+j, KB16*128 + r*128 + p] * XSCALE
        xFd = nc.dram_tensor("xF", [KT, KF8, MTILES * MT], f8,
                             kind="ExternalInput")
        # wF[p, r, n*512 + j] = W[n*512+j, KB16*128 + r*128 + p] / XSCALE
        wFd = nc.dram_tensor("wF", [KT, KF8, NTILES * NT], f8,
                             kind="ExternalInput")
    bD = nc.dram_tensor("bD", [MT, DOUT], f32, kind="ExternalInput")
    y = nc.dram_tensor("y", [TOKC, DOUT], ydt, kind="ExternalOutput")

    with tile.TileContext(nc) as tc:
        with (
            tc.tile_pool(name="xpool", bufs=1) as xpool,
            tc.tile_pool(name="bpool", bufs=1) as bpool,
            tc.tile_pool(name="opool", bufs=8) as opool,
            tc.tile_pool(name="psum", bufs=8, space="PSUM") as psum_pool,
        ):
            xB = xpool.tile([KT, MTILES * XW], bf16, name="xB", tag="xB")
            wB = xpool.tile([KT, NTILES * WW], bf16, name="wB", tag="wB")
            if FP8:
                xF = xpool.tile([KT, KF8, MTILES * MT], f8,
                                name="xF", tag="xF")
                wF = xpool.tile([KT, KF8, NTILES * NT], f8,
                                name="wF", tag="wF")
            bS = bpool.tile([MT, DOUT], f32, name="bS", tag="bS")

            # --- freight, ordered by first-use time -----------------------
            # Both queues interleave gate slices by exact first-use time;
            # group 0 gates on ~1.4 MB total instead of whole tensors.
            # sync (SP, ~2x the early bandwidth of scalar/Activation):
            nc.sync.dma_start(xB[:, 0:XW], xBd[:, 0:XW])          # x m0
            if FP8:
                QX = 8 * MT                                        # m0-7
                nc.sync.dma_start(xF[:, :, 0:QX], xFd[:, :, 0:QX])
            nc.sync.dma_start(xB[:, XW:4 * XW], xBd[:, XW:4 * XW])  # m1-3
            for c in (1, 2):
                nc.sync.dma_start(xB[:, c * 4 * XW:(c + 1) * 4 * XW],
                                  xBd[:, c * 4 * XW:(c + 1) * 4 * XW])
            if FP8:
                nc.sync.dma_start(xF[:, :, QX:MTILES * MT],        # m8-31
                                  xFd[:, :, QX:MTILES * MT])
            nc.sync.dma_start(xB[:, 3 * 4 * XW:4 * 4 * XW],
                              xBd[:, 3 * 4 * XW:4 * 4 * XW])
            nc.sync.dma_start(wB[:, WW:2 * WW], wBd[:, WW:2 * WW])  # W n1
            for c in range(4, 8):
                nc.sync.dma_start(xB[:, c * 4 * XW:(c + 1) * 4 * XW],
                                  xBd[:, c * 4 * XW:(c + 1) * 4 * XW])
            nc.sync.dma_start(bS[:, NT:DOUT], bD[:, NT:DOUT])
            for n in range(2, NTILES):
                nc.sync.dma_start(wB[:, n * WW:(n + 1) * WW],
                                  wBd[:, n * WW:(n + 1) * WW])
            # scalar: W n0 (k-halved), fp8 W n0-1, phase-0 bias, fp8 W
            # rest, then the outputs.
            nc.scalar.dma_start(wB[:, 0:WW // 2], wBd[:, 0:WW // 2])
            nc.scalar.dma_start(wB[:, WW // 2:WW], wBd[:, WW // 2:WW])
            if FP8:
                QW = 2 * NT                                        # n0-1
                nc.scalar.dma_start(wF[:, :, 0:QW], wFd[:, :, 0:QW])
            nc.scalar.dma_start(bS[:, 0:NT], bD[:, 0:NT])
            if FP8:
                nc.scalar.dma_start(wF[:, :, QW:NTILES * NT],      # n2-7
                                    wFd[:, :, QW:NTILES * NT])

            for n in range(NTILES):
                for m in range(MTILES):
                    acc = psum_pool.tile([MT, NT], mybir.dt.float32,
                                         name="acc", tag="acc")
                    for k in range(KB16):
                        nc.tensor.matmul(
                            acc[:],
                            xB[:, m * XW + k * KT:m * XW + (k + 1) * KT],
                            wB[:, n * WW + k * NT:n * WW + (k + 1) * NT],
                            start=(k == 0), stop=(not FP8 and k == KB16 - 1))
                    if FP8:
                        # DoubleRow accumulate-last: start=True on a DR
                        # matmul measurably corrupts the accumulation
                        nc.tensor.matmul(
                            acc[:],
                            xF[:, :, m * MT:(m + 1) * MT],
                            wF[:, :, n * NT:(n + 1) * NT],
                            start=False, stop=True,
                            perf_mode=mybir.MatmulPerfMode.DoubleRow)
                    ot = opool.tile([MT, NT], ydt, name="ot", tag="ot")
                    nc.vector.tensor_add(
                        ot[:], acc[:], bS[:, n * NT:(n + 1) * NT])
                    # final two tiles flush on the (by then idle) sync
                    # queue so the tail isn't serialized behind scalar
                    oeng = (nc.sync if (n == NTILES - 1 and m >= MTILES - 2)
                            else nc.scalar)
                    oeng.dma_start(
                        y[m * MT:(m + 1) * MT, n * NT:(n + 1) * NT], ot[:])

    nc.compile()
    return nc


def _install_neff_cache():
    """Disk-cache walrus NEFF compiles keyed on the BIR bytes."""
    if _CACHE.get("neff_cache_installed"):
        return
    _CACHE["neff_cache_installed"] = True
    import hashlib
    import shutil

    import concourse.bass2jax as bass2jax

    cache_dir = "/root/.neff_bir_cache"
    os.makedirs(cache_dir, exist_ok=True)
    orig = bass2jax.compile_bir_kernel

    def cached_compile(ant_bir_str, tmpdir, neff_name="file.neff", **kw):
        key = hashlib.sha256(
            ant_bir_str if isinstance(ant_bir_str, bytes)
            else ant_bir_str.encode()).hexdigest()
        hit = os.path.join(cache_dir, key + ".neff")
        dst = os.path.join(tmpdir, neff_name)
        if os.path.exists(hit):
            shutil.copyfile(hit, dst)
            return dst
        out = orig(ant_bir_str, tmpdir, neff_name=neff_name, **kw)
        try:
            shutil.copyfile(out, hit)
        except OSError:
            pass
        return out

    bass2jax.compile_bir_kernel = cached_compile


def _get_nc():
    if "nc" not in _CACHE:
        _install_neff_cache()
        _CACHE["nc"] = _build_nc()
    return _CACHE["nc"]


def kernel(input, weight, bias, num_experts_per_token):
    import ml_dtypes
    from concourse.bass_utils import run_bass_kernel_spmd

    bf16 = ml_dtypes.bfloat16
    f8 = ml_dtypes.float8_e4m3
    input = np.ascontiguousarray(np.asarray(input, dtype=np.float32))
    weight = np.ascontiguousarray(np.asarray(weight, dtype=np.float32))
    bias = np.ascontiguousarray(np.asarray(bias, dtype=np.float32))
    counts = np.asarray(num_experts_per_token).astype(np.int64)
    offsets = np.concatenate([[0], np.cumsum(counts)]).astype(np.int64)

    if counts.max() > TOKC:
        # capacity overflow (never hit with balanced routing): numpy fallback
        outs = []
        for i in range(E):
            xi = input[offsets[i]:offsets[i + 1]]
            outs.append(xi @ weight[i].T + bias[i])
        return np.concatenate(outs, axis=0)

    KB = KB16 * KT  # bf16 DIN columns
    MIN_NORMAL = 2.0 ** -6

    def q8(a):
        """Plain e4m3 RTN. The PE's DoubleRow path reads values at and
        below the min-normal as 0 (measured); promoting them to nearby
        normals makes hardware error worse, so leave them be."""
        return a.astype(f8)

    in_maps = []
    for i in range(E):
        xi = input[offsets[i]:offsets[i + 1]]  # [n_i, DIN]
        if xi.shape[0] < TOKC:
            xi = np.concatenate(
                [xi, np.zeros((TOKC - xi.shape[0], DIN), np.float32)], axis=0)
        wi = weight[i]
        m = {
            "xB": np.ascontiguousarray(
                xi[:, :KB].astype(bf16).reshape(MTILES, MT, KB16, KT)
                .transpose(3, 0, 2, 1).reshape(KT, MTILES * XW)),
            "wB": np.ascontiguousarray(
                wi[:, :KB].astype(bf16).reshape(NTILES, NT, KB16, KT)
                .transpose(3, 0, 2, 1).reshape(KT, NTILES * WW)),
            "bD": np.ascontiguousarray(
                np.broadcast_to(bias[i][None, :], (MT, DOUT))),
        }
        if FP8:
            m["xF"] = np.ascontiguousarray(
                q8(xi[:, KB:] * XSCALE)
                .reshape(MTILES, MT, KF8, KT)
                .transpose(3, 2, 0, 1).reshape(KT, KF8, MTILES * MT))
            m["wF"] = np.ascontiguousarray(
                q8(wi[:, KB:] * (1.0 / XSCALE))
                .reshape(NTILES, NT, KF8, KT)
                .transpose(3, 2, 0, 1).reshape(KT, KF8, NTILES * NT))
        in_maps.append(m)

    nc = _get_nc()
    trace = bool(int(os.environ.get("KERNEL_TRACE", "0")))
    res = run_bass_kernel_spmd(nc, in_maps, core_ids=list(range(NCORES)),
                               trace=trace)
    _CACHE["last_result"] = res

    out = np.empty((T, DOUT), dtype=np.float32)
    pos = 0
    for i in range(E):
        n_i = int(counts[i])
        out[pos:pos + n_i] = res.results[i]["y"][:n_i].astype(np.float32)
        pos += n_i
    return out


# revision 24
# speedup vs baseline: 1.2294x; 1.0033x over previous
"""MoE grouped-GEMM (8 experts) on 8 Trainium2 NeuronCores.

Problem: input [32768, 1024] routed contiguously to 8 experts (counts in
num_experts_per_token); expert i computes x_i @ W_i.T + b_i with
W [8, 4096, 1024], b [8, 4096]. Output [32768, 4096].

Sharding: expert-parallel, expert i <-> core i. Zero collectives; the host
slices per-expert token blocks, packs x and W into SBUF-tile layout, each
core runs a 4096x1024x4096 GEMM (+bias), host concatenates outputs.

Per-core kernel (tolerance budget 2e-2 lets precision buy speed):
  - k-tiles 0-5 (DIN 0:768) in bf16 at 1 PE cycle/row; k-tiles 6-7
    (DIN 768:1024) as one fp8-e4m3 DoubleRow matmul at 2x rate
    (x scaled down, W scaled up by the same factor so products land
    unscaled in the same fp32 PSUM accumulation). Measured rel err
    1.89e-2 (gate 2e-2), bit-exact reproducible across runs.
  - all operands SBUF-resident, each read from HBM exactly once
  - 8 n-phases of one 512-col W block; freight rides the two HWDGE
    queues statically ordered by first-use time with >=6 KB/partition
    segments (smaller segments collapse per-queue DMA throughput)
  - DVE fuses bias-add into the PSUM drain and casts outputs to bf16,
    halving the 67 MB output stream; the host casts back to fp32
"""

import sys

if "/opt/trn_rl_repo" not in sys.path:
    sys.path.insert(0, "/opt/trn_rl_repo")

import os

import numpy as np

E, T, DIN, DOUT = 8, 32768, 1024, 4096
NCORES = 8
TOKC = T // NCORES  # tokens per core (capacity)

KT = 128   # contraction tile (SBUF partitions)
MT = 128   # token tile (PSUM partitions)
NT = 512   # dout tile (one fp32 PSUM bank)
KTILES = DIN // KT    # 8
MTILES = TOKC // MT   # 32
NTILES = DOUT // NT   # 8

FP8 = bool(int(os.environ.get("KERNEL_FP8", "1")))
OUT_BF16 = bool(int(os.environ.get("KERNEL_OUT_BF16", "1")))
KB16 = 6 if FP8 else KTILES   # k-tiles carried in bf16
KF8 = KTILES - KB16           # k-tiles carried in fp8 (DoubleRow pair)
XSCALE = 0.1768               # fp8 x scale (W gets 1/XSCALE); hw-scanned
                              # minimum of rel err (flat 1.891-1.897e-2
                              # across 0.088-0.25)
XW = KB16 * KT                # bf16 x columns per m-tile: 768
WW = KB16 * NT                # bf16 W columns per n-block: 3072

_CACHE = {}


def _build_nc():
    import concourse.bacc as bacc
    import concourse.tile as tile
    import concourse.mybir as mybir

    nc = bacc.Bacc("TRN2", target_bir_lowering=False, debug=False,
                   num_devices=NCORES)

    f32 = mybir.dt.float32
    bf16 = mybir.dt.bfloat16
    f8 = mybir.dt.float8e4
    ydt = bf16 if OUT_BF16 else f32

    # xB[p, m*XW + k*128 + j] = x[m*128 + j, k*128 + p], k < KB16
    xBd = nc.dram_tensor("xB", [KT, MTILES * XW], bf16, kind="ExternalInput")
    # wB[p, n*WW + k*512 + j] = W[n*512 + j, k*128 + p], k < KB16
    wBd = nc.dram_tensor("wB", [KT, NTILES * WW], bf16, kind="ExternalInput")
    if FP8:
        # xF[p, r, m*128 + j] = x[m*128+j, KB16*128 + r*128 + p] * XSCALE
        xFd = nc.dram_tensor("xF", [KT, KF8, MTILES * MT], f8,
                             kind="ExternalInput")
        # wF[p, r, n*512 + j] = W[n*512+j, KB16*128 + r*128 + p] / XSCALE
        wFd = nc.dram_tensor("wF", [KT, KF8, NTILES * NT], f8,
                             kind="ExternalInput")
    bD = nc.dram_tensor("bD", [MT, DOUT], f32, kind="ExternalInput")
    y = nc.dram_tensor("y", [TOKC, DOUT], ydt, kind="ExternalOutput")

    with tile.TileContext(nc) as tc:
        with (
            tc.tile_pool(name="xpool", bufs=1) as xpool,
            tc.tile_pool(name="bpool", bufs=1) as bpool,
            tc.tile_pool(name="opool", bufs=12) as opool,
            tc.tile_pool(name="psum", bufs=8, space="PSUM") as psum_pool,
        ):
            xB = xpool.tile([KT, MTILES * XW], bf16, name="xB", tag="xB")
            wB = xpool.tile([KT, NTILES * WW], bf16, name="wB", tag="wB")
            if FP8:
                xF = xpool.tile([KT, KF8, MTILES * MT], f8,
                                name="xF", tag="xF")
                wF = xpool.tile([KT, KF8, NTILES * NT], f8,
                                name="wF", tag="wF")
            bS = bpool.tile([MT, DOUT], f32, name="bS", tag="bS")

            # --- freight, ordered by first-use time -----------------------
            # Both queues interleave gate slices by exact first-use time;
            # group 0 gates on ~1.4 MB total instead of whole tensors.
            # sync (SP, ~2x the early bandwidth of scalar/Activation):
            nc.sync.dma_start(xB[:, 0:XW], xBd[:, 0:XW])          # x m0
            if FP8:
                QX = 8 * MT                                        # m0-7
                nc.sync.dma_start(xF[:, :, 0:QX], xFd[:, :, 0:QX])
                QW = 2 * NT                                        # n0-1
                nc.sync.dma_start(wF[:, :, 0:QW], wFd[:, :, 0:QW])
            nc.sync.dma_start(xB[:, XW:4 * XW], xBd[:, XW:4 * XW])  # m1-3
            for c in (1, 2, 3):
                nc.sync.dma_start(xB[:, c * 4 * XW:(c + 1) * 4 * XW],
                                  xBd[:, c * 4 * XW:(c + 1) * 4 * XW])
            nc.sync.dma_start(wB[:, WW:2 * WW], wBd[:, WW:2 * WW])  # W n1
            for c in range(4, 8):
                nc.sync.dma_start(xB[:, c * 4 * XW:(c + 1) * 4 * XW],
                                  xBd[:, c * 4 * XW:(c + 1) * 4 * XW])
            nc.sync.dma_start(bS[:, NT:DOUT], bD[:, NT:DOUT])
            for n in range(2, NTILES):
                nc.sync.dma_start(wB[:, n * WW:(n + 1) * WW],
                                  wBd[:, n * WW:(n + 1) * WW])
            # scalar: W n0 (k-halved), phase-0 bias, late fp8 rests,
            # then the outputs.
            nc.scalar.dma_start(wB[:, 0:WW // 2], wBd[:, 0:WW // 2])
            nc.scalar.dma_start(wB[:, WW // 2:WW], wBd[:, WW // 2:WW])
            nc.scalar.dma_start(bS[:, 0:NT], bD[:, 0:NT])
            if FP8:
                nc.scalar.dma_start(xF[:, :, QX:MTILES * MT],      # m8-31
                                    xFd[:, :, QX:MTILES * MT])
                nc.scalar.dma_start(wF[:, :, QW:NTILES * NT],      # n2-7
                                    wFd[:, :, QW:NTILES * NT])

            for n in range(NTILES):
                for m in range(MTILES):
                    acc = psum_pool.tile([MT, NT], mybir.dt.float32,
                                         name="acc", tag="acc")
                    for k in range(KB16):
                        nc.tensor.matmul(
                            acc[:],
                            xB[:, m * XW + k * KT:m * XW + (k + 1) * KT],
                            wB[:, n * WW + k * NT:n * WW + (k + 1) * NT],
                            start=(k == 0), stop=(not FP8 and k == KB16 - 1))
                    if FP8:
                        # DoubleRow accumulate-last: start=True on a DR
                        # matmul measurably corrupts the accumulation
                        nc.tensor.matmul(
                            acc[:],
                            xF[:, :, m * MT:(m + 1) * MT],
                            wF[:, :, n * NT:(n + 1) * NT],
                            start=False, stop=True,
                            perf_mode=mybir.MatmulPerfMode.DoubleRow)
                    ot = opool.tile([MT, NT], ydt, name="ot", tag="ot")
                    nc.vector.tensor_add(
                        ot[:], acc[:], bS[:, n * NT:(n + 1) * NT])
                    # final two tiles flush on the (by then idle) sync
                    # queue so the tail isn't serialized behind scalar
                    oeng = (nc.sync if (n == NTILES - 1 and m >= MTILES - 2)
                            else nc.scalar)
                    oeng.dma_start(
                        y[m * MT:(m + 1) * MT, n * NT:(n + 1) * NT], ot[:])

    nc.compile()
    return nc


def _install_neff_cache():
    """Disk-cache walrus NEFF compiles keyed on the BIR bytes."""
    if _CACHE.get("neff_cache_installed"):
        return
    _CACHE["neff_cache_installed"] = True
    import hashlib
    import shutil

    import concourse.bass2jax as bass2jax

    cache_dir = "/root/.neff_bir_cache"
    os.makedirs(cache_dir, exist_ok=True)
    orig = bass2jax.compile_bir_kernel

    def cached_compile(ant_bir_str, tmpdir, neff_name="file.neff", **kw):
        key = hashlib.sha256(
            ant_bir_str if isinstance(ant_bir_str, bytes)
            else ant_bir_str.encode()).hexdigest()
        hit = os.path.join(cache_dir, key + ".neff")
        dst = os.path.join(tmpdir, neff_name)
        if os.path.exists(hit):
            shutil.copyfile(hit, dst)
            return dst
        out = orig(ant_bir_str, tmpdir, neff_name=neff_name, **kw)
        try:
            shutil.copyfile(out, hit)
        except OSError:
            pass
        return out

    bass2jax.compile_bir_kernel = cached_compile


def _get_nc():
    if "nc" not in _CACHE:
        _install_neff_cache()
        _CACHE["nc"] = _build_nc()
    return _CACHE["nc"]


def kernel(input, weight, bias, num_experts_per_token):
    import ml_dtypes
    from concourse.bass_utils import run_bass_kernel_spmd

    bf16 = ml_dtypes.bfloat16
    f8 = ml_dtypes.float8_e4m3
    input = np.ascontiguousarray(np.asarray(input, dtype=np.float32))
    weight = np.ascontiguousarray(np.asarray(weight, dtype=np.float32))
    bias = np.ascontiguousarray(np.asarray(bias, dtype=np.float32))
    counts = np.asarray(num_experts_per_token).astype(np.int64)
    offsets = np.concatenate([[0], np.cumsum(counts)]).astype(np.int64)

    if counts.max() > TOKC:
        # capacity overflow (never hit with balanced routing): numpy fallback
        outs = []
        for i in range(E):
            xi = input[offsets[i]:offsets[i + 1]]
            outs.append(xi @ weight[i].T + bias[i])
        return np.concatenate(outs, axis=0)

    KB = KB16 * KT  # bf16 DIN columns
    MIN_NORMAL = 2.0 ** -6

    def q8(a):
        """Plain e4m3 RTN. The PE's DoubleRow path reads values at and
        below the min-normal as 0 (measured); promoting them to nearby
        normals makes hardware error worse, so leave them be."""
        return a.astype(f8)

    in_maps = []
    for i in range(E):
        xi = input[offsets[i]:offsets[i + 1]]  # [n_i, DIN]
        if xi.shape[0] < TOKC:
            xi = np.concatenate(
                [xi, np.zeros((TOKC - xi.shape[0], DIN), np.float32)], axis=0)
        wi = weight[i]
        m = {
            "xB": np.ascontiguousarray(
                xi[:, :KB].astype(bf16).reshape(MTILES, MT, KB16, KT)
                .transpose(3, 0, 2, 1).reshape(KT, MTILES * XW)),
            "wB": np.ascontiguousarray(
                wi[:, :KB].astype(bf16).reshape(NTILES, NT, KB16, KT)
                .transpose(3, 0, 2, 1).reshape(KT, NTILES * WW)),
            "bD": np.ascontiguousarray(
                np.broadcast_to(bias[i][None, :], (MT, DOUT))),
        }
        if FP8:
            m["xF"] = np.ascontiguousarray(
                q8(xi[:, KB:] * XSCALE)
                .reshape(MTILES, MT, KF8, KT)
                .transpose(3, 2, 0, 1).reshape(KT, KF8, MTILES * MT))
            m["wF"] = np.ascontiguousarray(
                q8(wi[:, KB:] * (1.0 / XSCALE))
                .reshape(NTILES, NT, KF8, KT)
                .transpose(3, 2, 0, 1).reshape(KT, KF8, NTILES * NT))
        in_maps.append(m)

    nc = _get_nc()
    trace = bool(int(os.environ.get("KERNEL_TRACE", "0")))
    res = run_bass_kernel_spmd(nc, in_maps, core_ids=list(range(NCORES)),
                               trace=trace)
    _CACHE["last_result"] = res

    out = np.empty((T, DOUT), dtype=np.float32)
    pos = 0
    for i in range(E):
        n_i = int(counts[i])
        out[pos:pos + n_i] = res.results[i]["y"][:n_i].astype(np.float32)
        pos += n_i
    return out
